# revision 1
# baseline (speedup 1.0000x reference)
"""Trainium2 Bass kernel v2 for nn_AttModel (B=8, S=96, D=768, R=24, RSEQ=8, TAG=3).

Data-parallel over batch: core i handles sample i.

Per-core structure:
  - weights host-cast bf16 + packed (proj_W 2 DMAs, rel_W 1 DMA)
  - refine scan (24 seq softmax steps, f32, in PSUM score space) with the
    normalize folded into tiny [8,8] stationary rescales (G'=G*rinv rows,
    D'=diag(rinv)); wsum accumulates on the PE.
  - H = H0(b0) + AW.T-slice @ wsum (rank-8 correction); H0 feature-major
    chains + AW computed on the PE in the scan's shadow (emitted for scan
    steps >= 8, after the W halves have landed).
  - main loop: V tiles [128,384] bf16 = relu(ht + hh_i); DVE_MS tiles on
    DVE (tensor_scalar add+max), ACT_MS tiles on ACT reading PSUM-resident
    ht (ScalarE activation Relu, bias=hh col); 18 MMs accumulate
    out[72,384]; output DMA'd straight from PSUM.

PSUM budget (8 banks, 1 bank per pool buffer):
  pso 2 | psr 1 (5 ht slices packed in one bank) | scanb 1 (s|wsum|G) |
  pstmp 4 (transposes/AW chunks/H0 chains/Delta ring)
"""
import sys

sys.path.insert(0, "/opt/trn_rl_repo")

import numpy as np

S, D, H3 = 96, 768, 2304
R, RSEQ, TAG, C = 24, 8, 3, 72
B = 8
KT = D // 128           # 6 k-tiles over D
MT = H3 // 128          # 18 m-tiles over 3D
IGRP = 4
NG = S // IGRP          # 24 groups
NFREE = IGRP * S        # 384
SCALE = 1.0 / float(np.sqrt(np.float32(D)))

N_ACT = 5               # m-tiles produced on ACT (PSUM-resident ht)
ACT_MS = list(range(MT - N_ACT, MT))
DVE_MS = list(range(MT - N_ACT))
AWC = 384               # AW psum chunk width
SHADOW_START = 8        # first scan step that gets shadow PE work
OUT_PSUM_DMA = False    # bass dma_start rejects PSUM source


def build_nc(repeat: int = 1):
    import concourse.bass as bass
    from concourse import bacc, mybir
    import concourse.tile as tile
    from concourse.masks import make_identity

    f32 = mybir.dt.float32
    bf16 = mybir.dt.bfloat16
    AF = mybir.ActivationFunctionType
    ALU = mybir.AluOpType
    AX = mybir.AxisListType

    nc = bacc.Bacc()
    enc = nc.dram_tensor("enc", [S, D], f32, kind="ExternalInput")
    arel = nc.dram_tensor("arel", [RSEQ, D], f32, kind="ExternalInput")
    pwbf = nc.dram_tensor("pwbf", [128, 2 * KT * H3], bf16, kind="ExternalInput")
    projb = nc.dram_tensor("projb", [H3], f32, kind="ExternalInput")
    rwbf = nc.dram_tensor("rwbf", [128, MT * C], bf16, kind="ExternalInput")
    out = nc.dram_tensor("out", [C, S * S], f32, kind="ExternalOutput")

    with tile.TileContext(nc) as tc:
        with (
            tc.tile_pool(name="persist", bufs=1) as pp,
            tc.tile_pool(name="work", bufs=4) as wp,
            tc.tile_pool(name="vd", bufs=20) as vdp,
            tc.tile_pool(name="va", bufs=10) as vap,
            tc.tile_pool(name="pso", bufs=2, space="PSUM") as pso,
            tc.tile_pool(name="psr", bufs=1, space="PSUM") as psrp,
            tc.tile_pool(name="psss", bufs=1, space="PSUM") as psss,
            tc.tile_pool(name="pssw", bufs=1, space="PSUM") as pssw,
            tc.tile_pool(name="pstmp", bufs=3, space="PSUM") as pstmp,
        ):

            def body(_it=None):
                # ---------- loads ----------
                ident = pp.tile([128, 128], f32, tag="ident")
                make_identity(nc, ident[:])

                enc_nat = pp.tile([S, D], f32, tag="enc_nat")
                nc.sync.dma_start(enc_nat[:], enc[:])
                a_nat = pp.tile([RSEQ, D], f32, tag="a_nat")
                nc.sync.dma_start(a_nat[:], arel[:])
                pb_sb = pp.tile([128, MT], f32, tag="pb")
                nc.sync.dma_start(
                    pb_sb[:], projb.rearrange("(t p) -> p t", p=128)
                )
                rw_all = pp.tile([128, MT * C], bf16, tag="rw_all")
                nc.sync.dma_start(rw_all[:], rwbf[:])
                pw_h = pp.tile([128, KT * H3], bf16, tag="pw_h")
                nc.sync.dma_start(pw_h[:], pwbf[:, : KT * H3])
                pw_t = pp.tile([128, KT * H3], bf16, tag="pw_t")
                nc.sync.dma_start(pw_t[:], pwbf[:, KT * H3:])

                def pw(kt):
                    src = pw_h if kt < KT else pw_t
                    k = kt % KT
                    return src[:, k * H3:(k + 1) * H3]

                def rwr(m):
                    return rw_all[:, m * C:(m + 1) * C]

                # PSUM tiles: every open matmul chain owns a full bank
                # (start=True zeroes the whole 2KB zero-region).
                s_full = psss.tile([RSEQ, S], f32, tag="s", bufs=1)
                s_ps = s_full[:]
                w_full = pssw.tile([RSEQ, S], f32, tag="w", bufs=1)
                wsum_ps = w_full[:]
                # psr bank holds finalized ht data for ACT_MS; it is never a
                # matmul target (written by DVE tensor ops only).
                psr_all = psrp.tile([128, N_ACT * S], f32, tag="psr", bufs=1)
                psr_ht = {
                    m: psr_all[:, i * S:(i + 1) * S]
                    for i, m in enumerate(ACT_MS)
                }

                # ---------- transposes ----------
                bT_f, bT_b = [], []
                for k in range(KT):
                    ps = pstmp.tile([128, 384], f32, tag="tmp")
                    nc.tensor.transpose(
                        ps[:, :S],
                        enc_nat[:, k * 128:(k + 1) * 128], ident[:S, :S]
                    )
                    tf = pp.tile([128, S], f32, tag=f"bTf{k}")
                    nc.scalar.copy(tf[:], ps[:, :S])
                    tb = pp.tile([128, S], bf16, tag=f"bTb{k}")
                    nc.vector.tensor_scalar_mul(tb[:], ps[:, :S], 1.0)
                    bT_f.append(tf)
                    bT_b.append(tb)
                at_scl, at_bf = [], []
                at_raw = []
                for k in range(KT):
                    ps = pstmp.tile([128, 384], f32, tag="tmp")
                    nc.tensor.transpose(
                        ps[:, :RSEQ], a_nat[:, k * 128:(k + 1) * 128],
                        ident[:RSEQ, :RSEQ],
                    )
                    tr = pp.tile([128, RSEQ], f32, tag=f"atr{k}")
                    nc.scalar.copy(tr[:], ps[:, :RSEQ])
                    ts = pp.tile([128, RSEQ], f32, tag=f"ats{k}")
                    nc.scalar.mul(ts[:], ps[:, :RSEQ], SCALE)
                    tbf = pp.tile([128, RSEQ], bf16, tag=f"atb{k}")
                    nc.vector.tensor_scalar_mul(tbf[:], ps[:, :RSEQ], 1.0)
                    at_raw.append(tr)
                    at_scl.append(ts)
                    at_bf.append(tbf)

                # G = scale * A @ A.T (symmetric) [8, 8] via tmp ring
                gtmp = pstmp.tile([128, 384], f32, tag="tmp")
                for k in range(KT):
                    nc.tensor.matmul(
                        gtmp[:RSEQ, :RSEQ], at_scl[k][:], at_raw[k][:],
                        start=(k == 0), stop=(k == KT - 1),
                        skip_group_check=True,
                    )
                g_sb = pp.tile([RSEQ, RSEQ], f32, tag="g")
                nc.vector.tensor_scalar_mul(g_sb[:], gtmp[:RSEQ, :RSEQ], 1.0)
                i8 = pp.tile([RSEQ, RSEQ], f32, tag="i8")
                make_identity(nc, i8[:])

                # s0 = scale * A @ b0.T
                for k in range(KT):
                    nc.tensor.matmul(
                        s_ps, at_scl[k][:], bT_f[k][:],
                        start=(k == 0), stop=False, skip_group_check=True,
                    )

                # ---------- shadow PE work (AW + H0), emitted inside scan --
                aw_sb = pp.tile([RSEQ, 2 * H3], bf16, tag="aw")
                hh0 = [None] * MT
                ht0 = [None] * MT
                shadow = []

                def emit_aw(half, c0, cw):
                    def go():
                        ps = pstmp.tile([128, 384], f32, tag="tmp")
                        for k in range(KT):
                            nc.tensor.matmul(
                                ps[:RSEQ, :cw],
                                at_bf[k][:],
                                (pw_h if half == 0 else pw_t)[
                                    :, k * H3 + c0: k * H3 + c0 + cw
                                ],
                                start=(k == 0), stop=(k == KT - 1),
                                skip_group_check=True,
                            )
                        nc.vector.tensor_scalar_mul(
                            aw_sb[:, half * H3 + c0: half * H3 + c0 + cw],
                            ps[:RSEQ, :cw], 1.0,
                        )
                    return go

                def emit_h0h(m):
                    def go():
                        ps = pstmp.tile([128, 384], f32, tag="tmp")
                        msl = slice(m * 128, (m + 1) * 128)
                        for k in range(KT):
                            nc.tensor.matmul(
                                ps[:, :S], pw(k)[:, msl], bT_b[k][:],
                                start=(k == 0), stop=(k == KT - 1),
                                skip_group_check=True,
                            )
                        t = pp.tile([128, S], f32, tag=f"hh0_{m}")
                        if m % 2 == 0:
                            nc.scalar.activation(
                                t[:], ps[:, :S], AF.Identity,
                                bias=pb_sb[:, m:m + 1], scale=1.0,
                            )
                        else:
                            nc.vector.tensor_scalar(
                                t[:], ps[:, :S], pb_sb[:, m:m + 1], None,
                                op0=ALU.add,
                            )
                        hh0[m] = t
                    return go

                def emit_h0t(m):
                    def go():
                        msl = slice(m * 128, (m + 1) * 128)
                        ps = pstmp.tile([128, 384], f32, tag="tmp")
                        for k in range(KT):
                            nc.tensor.matmul(
                                ps[:, :S], pw(KT + k)[:, msl], bT_b[k][:],
                                start=(k == 0), stop=(k == KT - 1),
                                skip_group_check=True,
                            )
                        t = pp.tile([128, S], f32, tag=f"ht0_{m}")
                        if m % 2 == 0:
                            nc.scalar.copy(t[:], ps[:, :S])
                        else:
                            nc.vector.tensor_scalar_mul(
                                t[:], ps[:, :S], 1.0
                            )
                        ht0[m] = t
                    return go

                for half in range(2):
                    for c0 in range(0, H3, AWC):
                        shadow.append(emit_aw(half, c0, min(AWC, H3 - c0)))
                for m in range(MT):
                    shadow.append(emit_h0h(m))
                for m in range(MT):
                    shadow.append(emit_h0t(m))

                shadow_iter = iter(shadow)

                def run_shadow(n):
                    for _ in range(n):
                        thunk = next(shadow_iter, None)
                        if thunk is None:
                            return
                        thunk()

                # ---------- refine scan ----------
                nsh = len(shadow)  # 48
                steps_with_shadow = R - SHADOW_START
                per_step = -(-nsh // steps_with_shadow)  # ceil
                for t in range(R):
                    negmax = wp.tile([RSEQ, 1], f32, tag="negmax")
                    nc.vector.reduce_max(
                        negmax[:], s_ps, axis=AX.X, negate=True
                    )
                    u = wp.tile([RSEQ, S], f32, tag="u")
                    rs = wp.tile([RSEQ, 1], f32, tag="rs")
                    nc.scalar.activation(
                        u[:], s_ps, AF.Exp, bias=negmax[:], scale=1.0,
                        accum_out=rs[:],
                    )
                    rinv = wp.tile([RSEQ, 1], f32, tag="rinv")
                    nc.vector.reciprocal(rinv[:], rs[:])
                    gp = wp.tile([RSEQ, RSEQ], f32, tag="gp")
                    nc.vector.tensor_scalar_mul(gp[:], g_sb[:], rinv[:])
                    dp = wp.tile([RSEQ, RSEQ], f32, tag="dp")
                    nc.vector.tensor_scalar_mul(dp[:], i8[:], rinv[:])
                    nc.tensor.matmul(
                        wsum_ps, dp[:], u[:],
                        start=(t == 0), stop=(t == R - 1),
                        skip_group_check=True,
                    )
                    if t < R - 1:
                        nc.tensor.matmul(
                            s_ps, gp[:], u[:],
                            start=False, stop=(t == R - 2),
                            skip_group_check=True,
                        )
                    if t >= SHADOW_START:
                        run_shadow(per_step)
                run_shadow(nsh)

                wsum_bf = pp.tile([RSEQ, S], bf16, tag="wsum_bf")
                nc.vector.tensor_scalar_mul(wsum_bf[:], wsum_ps, 1.0)

                # ---------- Delta + finalize ----------
                hh = [None] * MT
                ht = [None] * MT
                for m in range(MT):
                    msl = slice(m * 128, (m + 1) * 128)
                    dps = pstmp.tile([128, 384], f32, tag="tmp")
                    nc.tensor.matmul(
                        dps[:, :S], aw_sb[:, msl], wsum_bf[:],
                        start=True, stop=True, skip_group_check=True,
                    )
                    th = pp.tile([128, S], f32, tag=f"hh{m}")
                    nc.vector.tensor_tensor(
                        th[:], hh0[m][:], dps[:, :S], op=ALU.add
                    )
                    hh[m] = th
                    tsl = slice(H3 + m * 128, H3 + (m + 1) * 128)
                    dpt = pstmp.tile([128, 384], f32, tag="tmp")
                    nc.tensor.matmul(
                        dpt[:, :S], aw_sb[:, tsl], wsum_bf[:],
                        start=True, stop=True, skip_group_check=True,
                    )
                    if m in ACT_MS:
                        nc.vector.tensor_tensor(
                            psr_ht[m], ht0[m][:], dpt[:, :S], op=ALU.add
                        )
                    else:
                        tt = pp.tile([128, S], bf16, tag=f"ht{m}")
                        nc.vector.tensor_tensor(
                            tt[:], ht0[m][:], dpt[:, :S], op=ALU.add
                        )
                        ht[m] = tt

                # ---------- pairwise main loop ----------
                # The staging copy of group g is emitted after group g+1's
                # V-production so the in-order DVE/ACT queues never stall
                # on the PE chain of the current group (pso ring = 2).
                pend = []

                def flush_out():
                    g0, ops0 = pend.pop(0)
                    ostg = wp.tile([C, NFREE], f32, tag="ostg")
                    if g0 % 2 == 0:
                        nc.scalar.copy(ostg[:], ops0[:])
                    else:
                        nc.vector.tensor_scalar_mul(ostg[:], ops0[:], 1.0)
                    nc.sync.dma_start(
                        out[:, g0 * NFREE:(g0 + 1) * NFREE], ostg[:]
                    )

                for ig in range(NG):
                    ops = pso.tile([C, NFREE], f32, tag="ops")
                    vtiles = {}
                    for m in ACT_MS:
                        v = vap.tile([128, NFREE], bf16, tag="va")
                        for ii in range(IGRP):
                            i = ig * IGRP + ii
                            nc.scalar.activation(
                                v[:, ii * S:(ii + 1) * S], psr_ht[m],
                                AF.Relu, bias=hh[m][:, i:i + 1], scale=1.0,
                            )
                        vtiles[m] = v
                    for m in DVE_MS:
                        v = vdp.tile([128, NFREE], bf16, tag="vd")
                        for ii in range(IGRP):
                            i = ig * IGRP + ii
                            nc.vector.tensor_scalar(
                                v[:, ii * S:(ii + 1) * S], ht[m][:],
                                hh[m][:, i:i + 1], 0.0,
                                op0=ALU.add, op1=ALU.max,
                            )
                        vtiles[m] = v
                    order = DVE_MS + ACT_MS
                    for j, m in enumerate(order):
                        nc.tensor.matmul(
                            ops[:], rwr(m), vtiles[m][:],
                            start=(j == 0), stop=(j == MT - 1),
                        )
                    pend.append((ig, ops))
                    if len(pend) > 1:
                        flush_out()
                while pend:
                    flush_out()

            if repeat == 1:
                body()
            else:
                with tc.For_i(0, repeat, 1) as it:
                    body(it)

    nc.finalize()
    return nc


_CACHED_NC = None


def _prep_in_maps(encoded_text, rel_types_encoded, proj_W, proj_b, rel_W):
    import ml_dtypes

    relw_perm = np.ascontiguousarray(
        np.asarray(rel_W, np.float32).reshape(H3, R, TAG)
        .transpose(0, 2, 1).reshape(H3, C)
    )
    rw_pack = np.zeros((128, MT * C), np.float32)
    for m in range(MT):
        rw_pack[:, m * C:(m + 1) * C] = relw_perm[m * 128:(m + 1) * 128, :]
    rw_pack = rw_pack.astype(ml_dtypes.bfloat16)

    pw = np.asarray(proj_W, np.float32)
    pw_pack = np.zeros((128, 2 * KT * H3), np.float32)
    for kt in range(2 * KT):
        pw_pack[:, kt * H3:(kt + 1) * H3] = pw[kt * 128:(kt + 1) * 128, :]
    pw_pack = pw_pack.astype(ml_dtypes.bfloat16)

    in_maps = []
    for i in range(B):
        in_maps.append({
            "enc": np.ascontiguousarray(encoded_text[i], dtype=np.float32),
            "arel": np.ascontiguousarray(
                rel_types_encoded[i], dtype=np.float32
            ),
            "pwbf": pw_pack,
            "projb": np.ascontiguousarray(proj_b, dtype=np.float32),
            "rwbf": rw_pack,
        })
    return in_maps


def _assemble(results, rel_b):
    outs = []
    for i in range(B):
        o = results[i]["out"].reshape(TAG, R, S, S)
        outs.append(o)
    full = np.stack(outs, axis=0).astype(np.float32)
    if np.any(rel_b):
        relb_perm = np.asarray(rel_b, dtype=np.float32).reshape(R, TAG).T
        full = full + relb_perm[None, :, :, None, None]
    return full


def kernel(encoded_text, rel_types_encoded, proj_W, proj_b, rel_W, rel_b):
    global _CACHED_NC
    from concourse.bass_utils import run_bass_kernel_spmd

    if _CACHED_NC is None:
        _CACHED_NC = build_nc(repeat=1)
    in_maps = _prep_in_maps(
        encoded_text, rel_types_encoded, proj_W, proj_b, rel_W
    )
    res = run_bass_kernel_spmd(_CACHED_NC, in_maps, list(range(B)))
    return _assemble(res.results, rel_b)



# revision 2
# speedup vs baseline: 1.0027x; 1.0027x over previous
"""Trainium2 Bass kernel v8 for nn_AttModel (B=8, S=96, D=768, R=24, RSEQ=8, TAG=3).

Data-parallel over batch: core i handles sample i.

v7 (on top of v6's pipelined body + max/P factorization):
  - relu(a+b) = max(a,-b)+b with the linear +hh term folded into the
    output as P = rw^T hh (tiny GEMM), added via the flush bias port.
  - The rank-8 refinement is applied to the TRANSPOSED encoder before
    the H GEMM: bT_corr[k] = bT[k] + A^T[k] @ wsum, so H = b_corr @ W
    comes out final -- no AW GEMM, no Delta MMs, no H0 staging buffers.
  - fin_m (slots 12-19, attached after m's last old-state read): hh/ht
    chains (12 MMs) into one PSUM bank, then stt -> hhn (DVE tiles) and
    jap TT -> HT4, or TT -> hh_fA/ht_bA (ACT tiles).
  - phase_a(t+1) (enc DMA, transposes, G, s0) runs in slots 13-16 so the
    next body's scan starts immediately.
  - scan: 2 steps/slot in slots 0-11; u/gp/dp in bf16 (full-rate PE MMs).
"""
import sys

sys.path.insert(0, "/opt/trn_rl_repo")

import numpy as np

S, D, H3 = 96, 768, 2304
R, RSEQ, TAG, C = 24, 8, 3, 72
B = 8
KT = D // 128
MT = H3 // 128
IGRP = 4
NG = S // IGRP           # 24
NFREE = IGRP * S         # 384
SCALE = 1.0 / float(np.sqrt(np.float32(D)))

N_DVE = 17
DVE_MS = list(range(N_DVE))
ACT_MS = list(range(N_DVE, MT))
N_ACT = len(ACT_MS)
WIDE = 3
NSUP = NG // WIDE        # 8
SUPI = WIDE * IGRP       # 12
NRING = 2                # v-tile ring depth (parity sup % 2)
NSLOT = NG


def build_nc(repeat: int = 1, use_pb: bool = False, scan_bf16: bool = True):
    import concourse.bass as bass
    from concourse import bacc, mybir
    import concourse.tile as tile
    from concourse.masks import make_identity

    f32 = mybir.dt.float32
    bf16 = mybir.dt.bfloat16
    AF = mybir.ActivationFunctionType
    ALU = mybir.AluOpType
    AX = mybir.AxisListType
    sdt = bf16 if scan_bf16 else f32

    nc = bacc.Bacc()
    enc = nc.dram_tensor("enc", [S, D], f32, kind="ExternalInput")
    arel = nc.dram_tensor("arel", [RSEQ, D], f32, kind="ExternalInput")
    pwbf = nc.dram_tensor("pwbf", [128, 2 * KT * H3], bf16, kind="ExternalInput")
    projb = nc.dram_tensor("projb", [H3], f32, kind="ExternalInput")
    rwbf = nc.dram_tensor("rwbf", [128, MT * C], bf16, kind="ExternalInput")
    sel4 = nc.dram_tensor("sel4", [96, 8 * NFREE], bf16, kind="ExternalInput")
    out = nc.dram_tensor("out", [C, S * S], f32, kind="ExternalOutput")

    with tile.TileContext(nc) as tc:
        with (
            tc.tile_pool(name="persist", bufs=1) as pp,
            tc.tile_pool(name="work", bufs=10) as wp,
            tc.tile_pool(name="pso", bufs=4, space="PSUM") as pso,
            tc.tile_pool(name="psss", bufs=1, space="PSUM") as psss,
            tc.tile_pool(name="pstmp", bufs=3, space="PSUM") as pstmp,
        ):
            # ---- once-only loads & constants ----
            ident = pp.tile([128, 128], f32, name="ident", tag="ident")
            make_identity(nc, ident[:])
            i8 = pp.tile([RSEQ, RSEQ], f32, name="i8", tag="i8")
            make_identity(nc, i8[:])
            rw_all = pp.tile([128, MT * C], bf16, name="rw_all", tag="rw_all")
            nc.sync.dma_start(rw_all[:], rwbf[:])
            pw_h = pp.tile([128, KT * H3], bf16, name="pw_h", tag="pw_h")
            nc.sync.dma_start(pw_h[:], pwbf[:, : KT * H3])
            pw_t = pp.tile([128, KT * H3], bf16, name="pw_t", tag="pw_t")
            nc.sync.dma_start(pw_t[:], pwbf[:, KT * H3:])
            pb_sb = pp.tile([128, MT], f32, name="pb", tag="pb")
            sel_sb = pp.tile([96, 8 * NFREE], bf16, name="sel_sb", tag="sel_sb")
            nc.sync.dma_start(sel_sb[:], sel4[:])
            if use_pb:
                nc.sync.dma_start(
                    pb_sb[:], projb.rearrange("(t p) -> p t", p=128)
                )

            def rwr(m):
                return rw_all[:, m * C:(m + 1) * C]

            # ---- persistent per-iteration state ----
            enc_nat = pp.tile([S, D], f32, name="enc_nat", tag="enc_nat")
            a_nat = pp.tile([RSEQ, D], f32, name="a_nat", tag="a_nat")
            a_bf = pp.tile([RSEQ, D], bf16, name="a_bf", tag="a_bf")
            bT_f = [pp.tile([128, S], f32, name=f"bTf{k}", tag=f"bTf{k}")
                    for k in range(KT)]
            at_raw = [pp.tile([128, RSEQ], f32, name=f"atr{k}", tag=f"atr{k}")
                      for k in range(KT)]
            at_scl = [pp.tile([128, RSEQ], f32, name=f"ats{k}", tag=f"ats{k}")
                      for k in range(KT)]
            g_sb = pp.tile([RSEQ, RSEQ], f32, name="g", tag="g")
            bt_c = [pp.tile([128, S], bf16, name=f"btc{k}", tag=f"btc{k}")
                    for k in range(KT)]
            wsum_bf = pp.tile([RSEQ, S], bf16, name="wsum_bf", tag="wsum_bf")
            hhn_bf = pp.tile([128, N_DVE * S], bf16, name="hhn", tag="hhn")
            ht4 = [pp.tile([128, NFREE], bf16, name=f"ht4_{m}", tag=f"ht4_{m}")
                   for m in DVE_MS]
            hh_fA = pp.tile([128, N_ACT * S], f32, name="hh_fA", tag="hh_fA")
            ht_bA = pp.tile([128, N_ACT * S], bf16, name="ht_bA", tag="ht_bA")
            vd_t = [
                [pp.tile([128, WIDE * NFREE], bf16, name=f"vd{m}_{p}",
                         tag=f"vd{m}_{p}") for p in range(NRING)]
                for m in DVE_MS
            ]
            va_t = [
                [pp.tile([128, WIDE * NFREE], bf16, name=f"va{a}_{p}",
                         tag=f"va{a}_{p}") for p in range(NRING)]
                for a in range(N_ACT)
            ]
            ostg_t = [pp.tile([C, NFREE], f32, name=f"ostg{r_}", tag=f"ostg{r_}")
                      for r_ in range(4)]
            pT_sb = pp.tile([S, C], bf16, name="pT_sb", tag="pT_sb")

            # s and wsum share one PSUM bank: the s0 chain's start=True
            # zeroes the whole bank, so all wsum MMs accumulate start=False.
            sw_full = psss.tile([RSEQ, 2 * S], f32, name="sw", tag="sw", bufs=1)
            s_ps = sw_full[:, :S]
            wsum_ps = sw_full[:, S:]

            # ---- emission helpers ----
            def phase_a():
                """Loads + transposes + G + s0 for the NEXT scan."""
                nc.sync.dma_start(enc_nat[:], enc[:])
                nc.sync.dma_start(a_nat[:], arel[:])
                nc.vector.tensor_scalar_mul(a_bf[:], a_nat[:], 1.0)
                for k in range(KT):
                    ps = pstmp.tile([128, NFREE], f32, name="tmp", tag="tmp")
                    nc.tensor.transpose(
                        ps[:, :S], enc_nat[:, k * 128:(k + 1) * 128],
                        ident[:S, :S],
                    )
                    nc.scalar.copy(bT_f[k][:], ps[:, :S])
                for k in range(KT):
                    ps = pstmp.tile([128, NFREE], f32, name="tmp", tag="tmp")
                    nc.tensor.transpose(
                        ps[:, :RSEQ], a_nat[:, k * 128:(k + 1) * 128],
                        ident[:RSEQ, :RSEQ],
                    )
                    nc.scalar.copy(at_raw[k][:], ps[:, :RSEQ])
                    nc.scalar.mul(at_scl[k][:], ps[:, :RSEQ], SCALE)
                gtmp = pstmp.tile([128, NFREE], f32, name="tmp", tag="tmp")
                for k in range(KT):
                    nc.tensor.matmul(
                        gtmp[:RSEQ, :RSEQ], at_scl[k][:], at_raw[k][:],
                        start=(k == 0), stop=(k == KT - 1),
                        skip_group_check=True,
                    )
                nc.vector.tensor_scalar_mul(g_sb[:], gtmp[:RSEQ, :RSEQ], 1.0)
                for k in range(KT):
                    nc.tensor.matmul(
                        s_ps, at_scl[k][:], bT_f[k][:],
                        start=(k == 0), stop=False, skip_group_check=True,
                    )

            nmx_prev = {"t": None}

            def scan_step(t):
                negmax = wp.tile([RSEQ, 1], f32, name="negmax", tag="negmax")
                if t % 2 == 1 and nmx_prev["t"] is not None:
                    nc.vector.tensor_scalar_add(
                        negmax[:], nmx_prev["t"][:], -12.0
                    )
                else:
                    nc.vector.reduce_max(
                        negmax[:], s_ps, axis=AX.X, negate=True
                    )
                nmx_prev["t"] = negmax
                u = wp.tile([RSEQ, S], sdt, name="u", tag="u")
                rs = wp.tile([RSEQ, 1], f32, name="rs", tag="rs")
                nc.scalar.activation(
                    u[:], s_ps, AF.Exp, bias=negmax[:], scale=1.0,
                    accum_out=rs[:],
                )
                rinv = wp.tile([RSEQ, 1], f32, name="rinv", tag="rinv")
                nc.vector.reciprocal(rinv[:], rs[:])
                gp = wp.tile([RSEQ, RSEQ], sdt, name="gp", tag="gp")
                nc.scalar.mul(gp[:], g_sb[:], rinv[:])
                dp = wp.tile([RSEQ, RSEQ], sdt, name="dp", tag="dp")
                nc.scalar.mul(dp[:], i8[:], rinv[:])
                nc.tensor.matmul(
                    wsum_ps, dp[:], u[:],
                    start=False, stop=(t == R - 1), skip_group_check=True,
                )
                if t < R - 1:
                    nc.tensor.matmul(
                        s_ps, gp[:], u[:],
                        start=False, stop=(t == R - 2), skip_group_check=True,
                    )

            def emit_wsum_bf():
                nc.vector.tensor_scalar_mul(wsum_bf[:], wsum_ps, 1.0)

            def emit_corr():
                """bt_c[k] = bT[k] + A^T[k] @ wsum  (bf16 out)."""
                for k0 in range(0, KT, 3):
                    ps = pstmp.tile([128, NFREE], f32, name="tmp", tag="tmp")
                    for kk in range(3):
                        k = k0 + kk
                        nc.tensor.matmul(
                            ps[:, kk * S:(kk + 1) * S],
                            a_bf[:, k * 128:(k + 1) * 128], wsum_bf[:],
                            start=(kk == 0), stop=(kk == 2),
                            skip_group_check=True,
                        )
                    for kk in range(3):
                        k = k0 + kk
                        nc.vector.tensor_tensor(
                            bt_c[k][:], bT_f[k][:], ps[:, kk * S:(kk + 1) * S],
                            op=ALU.add,
                        )

            def fin_m(m):
                """H chains for m on corrected b + fold into V-prod state."""
                ps = pstmp.tile([128, NFREE], f32, name="tmp", tag="tmp")
                msl = slice(m * 128, (m + 1) * 128)
                for k in range(KT):
                    nc.tensor.matmul(
                        ps[:, :S], pw_h[:, k * H3:(k + 1) * H3][:, msl],
                        bt_c[k][:],
                        start=(k == 0), stop=False, skip_group_check=True,
                    )
                for k in range(KT):
                    nc.tensor.matmul(
                        ps[:, S:2 * S], pw_t[:, k * H3:(k + 1) * H3][:, msl],
                        bt_c[k][:],
                        start=False, stop=(k == KT - 1),
                        skip_group_check=True,
                    )
                if use_pb:
                    nc.scalar.activation(
                        ps[:, :S], ps[:, :S], AF.Identity,
                        bias=pb_sb[:, m:m + 1], scale=1.0,
                    )
                if m < N_DVE:
                    nc.scalar.mul(hhn_bf[:, m * S:(m + 1) * S], ps[:, :S], -1.0)
                    vjap = ht4[m][:].rearrange("p (j r) -> p j r", r=4)
                    in0 = ps[:, S:2 * S].unsqueeze(2).broadcast_to([128, S, 4])
                    nc.scalar.copy(vjap, in0)
                else:
                    a = m - N_DVE
                    asl = slice(a * S, (a + 1) * S)
                    nc.scalar.copy(hh_fA[:, asl], ps[:, :S])
                    nc.scalar.copy(ht_bA[:, asl], ps[:, S:2 * S])

            def pair_thunk(m, sup, fin_after):
                def go():
                    vt = vd_t[m][sup % NRING]
                    i0 = sup * SUPI
                    in0 = (
                        ht4[m][:].rearrange("p (j r) -> p j r", r=4)
                        .unsqueeze(1).broadcast_to([128, WIDE, S, 4])
                    )
                    hq = hhn_bf[:, m * S + i0: m * S + i0 + SUPI].rearrange(
                        "p (g i) -> p g i", g=WIDE
                    )
                    in1 = hq.unsqueeze(2).broadcast_to([128, WIDE, S, 4])
                    vv = vt[:].rearrange(
                        "p (g j i) -> p g j i", g=WIDE, i=IGRP
                    )
                    nc.vector.tensor_tensor(vv, in0, in1, op=ALU.max)
                    if fin_after:
                        fin_m(m)
                return go

            def act_thunk(a, sup, gl, fin_after):
                def go():
                    vt = va_t[a][sup % NRING]
                    for ii in range(IGRP):
                        i = sup * SUPI + gl * IGRP + ii
                        dst = vt[:, gl * NFREE:(gl + 1) * NFREE].rearrange(
                            "p (j r) -> p j r", r=4
                        )[:, :, ii]
                        nc.scalar.activation(
                            dst, ht_bA[:, a * S:(a + 1) * S], AF.Relu,
                            bias=hh_fA[:, a * S + i: a * S + i + 1], scale=1.0,
                        )
                    if fin_after:
                        fin_m(N_DVE + a)
                return go

            def make_prod_units(sup, with_fin):
                units = []
                for m in DVE_MS:
                    units.append(pair_thunk(m, sup, with_fin))
                for a in range(N_ACT):
                    for gl in range(WIDE):
                        units.append(
                            act_thunk(a, sup, gl, with_fin and gl == WIDE - 1)
                        )
                return units

            pend = []

            def chain(g):
                sup, gl = g // WIDE, g % WIDE
                ops = pso.tile([C, NFREE], f32, name="ops", tag="ops")
                mlist = DVE_MS + ACT_MS
                for j, m in enumerate(mlist):
                    vt = (vd_t[m][sup % NRING] if m < N_DVE
                          else va_t[m - N_DVE][sup % NRING])
                    nc.tensor.matmul(
                        ops[:], rwr(m), vt[:, gl * NFREE:(gl + 1) * NFREE],
                        start=(j == 0), stop=False,
                    )
                # + P[c, i] broadcast over j via constant selector
                base = (g // 8) * 32
                q = g % 8
                nc.tensor.matmul(
                    ops[:], pT_sb[base:base + 32, :],
                    sel_sb[base:base + 32, q * NFREE:(q + 1) * NFREE],
                    start=False, stop=True,
                )
                pend.append((g, ops))

            def flush_one():
                g0, ops0 = pend.pop(0)
                ostg = ostg_t[g0 % 4]
                src = ops0[:].rearrange("c (j i) -> c i j", i=IGRP)
                dst = ostg[:].rearrange("c (i j) -> c i j", j=S)
                nc.scalar.copy(dst, src)
                nc.sync.dma_start(
                    out[:, g0 * NFREE:(g0 + 1) * NFREE], ostg[:]
                )

            def emit_p():
                # pT[i, c] = -(hhn^T rw)[i, c] = (rw^T hh)^T, DVE-owned m only
                ps = pstmp.tile([128, NFREE], f32, name="tmp", tag="tmp")
                for j, m in enumerate(DVE_MS):
                    nc.tensor.matmul(
                        ps[:S, :C], hhn_bf[:, m * S:(m + 1) * S], rwr(m),
                        start=(j == 0), stop=(j == N_DVE - 1),
                        skip_group_check=True,
                    )
                nc.vector.tensor_scalar_mul(pT_sb[:], ps[:S, :C], -1.0)

            def emit_from(q, n):
                for _ in range(n):
                    if q:
                        q.pop(0)()

            def emit_body(do_scan, prod_next_sup0):
                # super w produced in slots [max(0, 2w-4) .. 2w-1]; ring-3
                # parity guarantees writes land after the last chain read of
                # super w-3 (slots 2w-6, 2w-5).
                prod_qs = {
                    w: make_prod_units(w, with_fin=(do_scan and w == NSUP - 1))
                    for w in range(1, NSUP)
                }
                windows = {
                    w: list(range(3 * (w - 1), 3 * w))
                    for w in range(1, NSUP)
                }
                quota = {}
                for w, sl in windows.items():
                    n = len(prod_qs[w])
                    base = n // len(sl)
                    ext = n - base * len(sl)
                    for idx, s_ in enumerate(sl):
                        quota[(w, s_)] = base + (1 if idx < ext else 0)
                sup0_q = make_prod_units(0, False) if prod_next_sup0 else []
                step_i = {"n": 0}
                for slot in range(NSLOT):
                    if do_scan and slot == 12:
                        emit_wsum_bf()
                        emit_corr()
                    if do_scan and slot == 13:
                        phase_a()
                    halves = ([], [])
                    for w in range(1, NSUP):
                        q = quota.get((w, slot))
                        if q:
                            halves[0].append((prod_qs[w], (q + 1) // 2))
                            halves[1].append((prod_qs[w], q // 2))
                    if do_scan and slot < 12:
                        scan_step(step_i["n"])
                        step_i["n"] += 1
                    for qq, n in halves[0]:
                        emit_from(qq, n)
                    if do_scan and slot < 12:
                        scan_step(step_i["n"])
                        step_i["n"] += 1
                    for qq, n in halves[1]:
                        emit_from(qq, n)
                    if prod_next_sup0 and slot >= 21:
                        emit_from(sup0_q, 7)
                    chain(slot)
                    if len(pend) > 2:
                        flush_one()
                for w in range(1, NSUP):
                    emit_from(prod_qs[w], 99)
                if prod_next_sup0:
                    emit_from(sup0_q, 99)
                while pend:
                    flush_one()
                if do_scan:
                    emit_p()

            def emit_prologue():
                phase_a()
                for t in range(R):
                    scan_step(t)
                emit_wsum_bf()
                emit_corr()
                for m in range(MT):
                    fin_m(m)
                emit_p()
                phase_a()
                for th in make_prod_units(0, False):
                    th()

            emit_prologue()
            n_bodies = repeat - 1
            if n_bodies >= 4:
                with tc.For_i(0, n_bodies // 4, 1):
                    for _ in range(4):
                        emit_body(do_scan=True, prod_next_sup0=True)
            for _ in range(n_bodies % 4):
                emit_body(do_scan=True, prod_next_sup0=True)
            emit_body(do_scan=False, prod_next_sup0=False)

    nc.finalize()
    return nc


_CACHED = {}


def _prep_in_maps(encoded_text, rel_types_encoded, proj_W, proj_b, rel_W):
    import ml_dtypes

    relw_perm = np.ascontiguousarray(
        np.asarray(rel_W, np.float32).reshape(H3, R, TAG)
        .transpose(0, 2, 1).reshape(H3, C)
    )
    rw_pack = np.zeros((128, MT * C), np.float32)
    for m in range(MT):
        rw_pack[:, m * C:(m + 1) * C] = relw_perm[m * 128:(m + 1) * 128, :]
    rw_pack = rw_pack.astype(ml_dtypes.bfloat16)

    pw = np.asarray(proj_W, np.float32)
    pw_pack = np.zeros((128, 2 * KT * H3), np.float32)
    for kt in range(2 * KT):
        pw_pack[:, kt * H3:(kt + 1) * H3] = pw[kt * 128:(kt + 1) * 128, :]
    pw_pack = pw_pack.astype(ml_dtypes.bfloat16)

    sel = np.zeros((96, 8 * NFREE), np.float32)
    for base in (0, 32, 64):
        for q in range(8):
            for j in range(S):
                for ii in range(IGRP):
                    sel[base + q * IGRP + ii,
                        q * NFREE + j * IGRP + ii] = 1.0
    sel = sel.astype(ml_dtypes.bfloat16)

    in_maps = []
    for i in range(B):
        in_maps.append({
            "enc": np.ascontiguousarray(encoded_text[i], dtype=np.float32),
            "arel": np.ascontiguousarray(
                rel_types_encoded[i], dtype=np.float32
            ),
            "pwbf": pw_pack,
            "projb": np.ascontiguousarray(proj_b, dtype=np.float32),
            "rwbf": rw_pack,
            "sel4": sel,
        })
    return in_maps


def _assemble(results, rel_b):
    outs = []
    for i in range(B):
        o = results[i]["out"].reshape(TAG, R, S, S)
        outs.append(o)
    full = np.stack(outs, axis=0).astype(np.float32)
    if np.any(rel_b):
        relb_perm = np.asarray(rel_b, dtype=np.float32).reshape(R, TAG).T
        full = full + relb_perm[None, :, :, None, None]
    return full


def kernel(encoded_text, rel_types_encoded, proj_W, proj_b, rel_W, rel_b):
    from concourse.bass_utils import run_bass_kernel_spmd

    use_pb = bool(np.any(np.asarray(proj_b)))
    key = (1, use_pb)
    if key not in _CACHED:
        _CACHED[key] = build_nc(repeat=1, use_pb=use_pb)
    in_maps = _prep_in_maps(
        encoded_text, rel_types_encoded, proj_W, proj_b, rel_W
    )
    res = run_bass_kernel_spmd(_CACHED[key], in_maps, list(range(B)))
    return _assemble(res.results, rel_b)


# revision 3
# speedup vs baseline: 1.0454x; 1.0425x over previous
"""Trainium2 Bass kernel v8 for nn_AttModel (B=8, S=96, D=768, R=24, RSEQ=8, TAG=3).

Data-parallel over batch: core i handles sample i.

v7 (on top of v6's pipelined body + max/P factorization):
  - relu(a+b) = max(a,-b)+b with the linear +hh term folded into the
    output as P = rw^T hh (tiny GEMM), added via the flush bias port.
  - The rank-8 refinement is applied to the TRANSPOSED encoder before
    the H GEMM: bT_corr[k] = bT[k] + A^T[k] @ wsum, so H = b_corr @ W
    comes out final -- no AW GEMM, no Delta MMs, no H0 staging buffers.
  - fin_m (slots 12-19, attached after m's last old-state read): hh/ht
    chains (12 MMs) into one PSUM bank, then stt -> hhn (DVE tiles) and
    jap TT -> HT4, or TT -> hh_fA/ht_bA (ACT tiles).
  - phase_a(t+1) (enc DMA, transposes, G, s0) runs in slots 13-16 so the
    next body's scan starts immediately.
  - scan: 2 steps/slot in slots 0-11; u/gp/dp in bf16 (full-rate PE MMs).
"""
import sys

sys.path.insert(0, "/opt/trn_rl_repo")

import numpy as np

S, D, H3 = 96, 768, 2304
R, RSEQ, TAG, C = 24, 8, 3, 72
B = 8
KT = D // 128
MT = H3 // 128
IGRP = 4
NG = S // IGRP           # 24
NFREE = IGRP * S         # 384
SCALE = 1.0 / float(np.sqrt(np.float32(D)))

N_DVE = 17
DVE_MS = list(range(N_DVE))
ACT_MS = list(range(N_DVE, MT))
N_ACT = len(ACT_MS)
WIDE = 3
NSUP = NG // WIDE        # 8
SUPI = WIDE * IGRP       # 12
NRING = 2                # v-tile ring depth (parity sup % 2)
NSLOT = NG


def build_nc(repeat: int = 1, use_pb: bool = False, scan_bf16: bool = True):
    import concourse.bass as bass
    from concourse import bacc, mybir
    import concourse.tile as tile
    from concourse.masks import make_identity

    f32 = mybir.dt.float32
    bf16 = mybir.dt.bfloat16
    AF = mybir.ActivationFunctionType
    ALU = mybir.AluOpType
    AX = mybir.AxisListType
    sdt = bf16 if scan_bf16 else f32

    nc = bacc.Bacc()
    enc = nc.dram_tensor("enc", [S, D], f32, kind="ExternalInput")
    arel = nc.dram_tensor("arel", [RSEQ, D], f32, kind="ExternalInput")
    pwbf = nc.dram_tensor("pwbf", [128, 2 * KT * H3], bf16, kind="ExternalInput")
    projb = nc.dram_tensor("projb", [H3], f32, kind="ExternalInput")
    rwbf = nc.dram_tensor("rwbf", [128, MT * C], bf16, kind="ExternalInput")
    sel4 = nc.dram_tensor("sel4", [96, 8 * NFREE], bf16, kind="ExternalInput")
    out = nc.dram_tensor("out", [C, S * S], f32, kind="ExternalOutput")

    with tile.TileContext(nc) as tc:
        with (
            tc.tile_pool(name="persist", bufs=1) as pp,
            tc.tile_pool(name="work", bufs=10) as wp,
            tc.tile_pool(name="pso", bufs=4, space="PSUM") as pso,
            tc.tile_pool(name="psss", bufs=1, space="PSUM") as psss,
            tc.tile_pool(name="pstmp", bufs=3, space="PSUM") as pstmp,
        ):
            # ---- once-only loads & constants ----
            ident = pp.tile([128, 128], f32, name="ident", tag="ident")
            make_identity(nc, ident[:])
            i8 = pp.tile([RSEQ, RSEQ], f32, name="i8", tag="i8")
            make_identity(nc, i8[:])
            rw_all = pp.tile([128, MT * C], bf16, name="rw_all", tag="rw_all")
            nc.sync.dma_start(rw_all[:], rwbf[:])
            pw_h = pp.tile([128, KT * H3], bf16, name="pw_h", tag="pw_h")
            nc.sync.dma_start(pw_h[:], pwbf[:, : KT * H3])
            pw_t = pp.tile([128, KT * H3], bf16, name="pw_t", tag="pw_t")
            nc.sync.dma_start(pw_t[:], pwbf[:, KT * H3:])
            pb_sb = pp.tile([128, MT], f32, name="pb", tag="pb")
            sel_sb = pp.tile([96, 8 * NFREE], bf16, name="sel_sb", tag="sel_sb")
            nc.sync.dma_start(sel_sb[:], sel4[:])
            if use_pb:
                nc.sync.dma_start(
                    pb_sb[:], projb.rearrange("(t p) -> p t", p=128)
                )

            def rwr(m):
                return rw_all[:, m * C:(m + 1) * C]

            # ---- persistent per-iteration state ----
            enc_nat = pp.tile([S, D], f32, name="enc_nat", tag="enc_nat")
            a_nat = pp.tile([RSEQ, D], f32, name="a_nat", tag="a_nat")
            a_bf = pp.tile([RSEQ, D], bf16, name="a_bf", tag="a_bf")
            bT_f = [pp.tile([128, S], f32, name=f"bTf{k}", tag=f"bTf{k}")
                    for k in range(KT)]
            at_raw = [pp.tile([128, RSEQ], f32, name=f"atr{k}", tag=f"atr{k}")
                      for k in range(KT)]
            at_scl = [pp.tile([128, RSEQ], f32, name=f"ats{k}", tag=f"ats{k}")
                      for k in range(KT)]
            g_sb = pp.tile([RSEQ, RSEQ], f32, name="g", tag="g")
            bt_c = [pp.tile([128, S], bf16, name=f"btc{k}", tag=f"btc{k}")
                    for k in range(KT)]
            wsum_bf = pp.tile([RSEQ, S], bf16, name="wsum_bf", tag="wsum_bf")
            hhn_bf = pp.tile([128, N_DVE * S], bf16, name="hhn", tag="hhn")
            ht4 = [pp.tile([128, NFREE], bf16, name=f"ht4_{m}", tag=f"ht4_{m}")
                   for m in DVE_MS]
            hh_fA = pp.tile([128, N_ACT * S], f32, name="hh_fA", tag="hh_fA")
            ht_bA = pp.tile([128, N_ACT * S], bf16, name="ht_bA", tag="ht_bA")
            vd_t = [
                [pp.tile([128, WIDE * NFREE], bf16, name=f"vd{m}_{p}",
                         tag=f"vd{m}_{p}") for p in range(NRING)]
                for m in DVE_MS
            ]
            va_t = [
                [pp.tile([128, WIDE * NFREE], bf16, name=f"va{a}_{p}",
                         tag=f"va{a}_{p}") for p in range(NRING)]
                for a in range(N_ACT)
            ]
            ostg_t = [pp.tile([C, NFREE], f32, name=f"ostg{r_}", tag=f"ostg{r_}")
                      for r_ in range(4)]
            pT_sb = pp.tile([S, C], bf16, name="pT_sb", tag="pT_sb")

            # s and wsum share one PSUM bank: the s0 chain's start=True
            # zeroes the whole bank, so all wsum MMs accumulate start=False.
            sw_full = psss.tile([RSEQ, 2 * S], f32, name="sw", tag="sw", bufs=1)
            s_ps = sw_full[:, :S]
            wsum_ps = sw_full[:, S:]

            # ---- emission helpers ----
            def phase_a():
                """Loads + transposes + G + s0 for the NEXT scan."""
                nc.sync.dma_start(enc_nat[:], enc[:])
                nc.sync.dma_start(a_nat[:], arel[:])
                nc.vector.tensor_scalar_mul(a_bf[:], a_nat[:], 1.0)
                for k in range(KT):
                    ps = pstmp.tile([128, NFREE], f32, name="tmp", tag="tmp")
                    nc.tensor.transpose(
                        ps[:, :S], enc_nat[:, k * 128:(k + 1) * 128],
                        ident[:S, :S],
                    )
                    nc.scalar.copy(bT_f[k][:], ps[:, :S])
                for k in range(KT):
                    ps = pstmp.tile([128, NFREE], f32, name="tmp", tag="tmp")
                    nc.tensor.transpose(
                        ps[:, :RSEQ], a_nat[:, k * 128:(k + 1) * 128],
                        ident[:RSEQ, :RSEQ],
                    )
                    nc.scalar.copy(at_raw[k][:], ps[:, :RSEQ])
                    nc.scalar.mul(at_scl[k][:], ps[:, :RSEQ], SCALE)
                gtmp = pstmp.tile([128, NFREE], f32, name="tmp", tag="tmp")
                for k in range(KT):
                    nc.tensor.matmul(
                        gtmp[:RSEQ, :RSEQ], at_scl[k][:], at_raw[k][:],
                        start=(k == 0), stop=(k == KT - 1),
                        skip_group_check=True,
                    )
                nc.vector.tensor_scalar_mul(g_sb[:], gtmp[:RSEQ, :RSEQ], 1.0)
                for k in range(KT):
                    nc.tensor.matmul(
                        s_ps, at_scl[k][:], bT_f[k][:],
                        start=(k == 0), stop=False, skip_group_check=True,
                    )

            nmx_prev = {"t": None}

            def scan_step(t):
                negmax = wp.tile([RSEQ, 1], f32, name="negmax", tag="negmax")
                if t % 4 != 0 and nmx_prev["t"] is not None:
                    nc.vector.tensor_scalar_add(
                        negmax[:], nmx_prev["t"][:], -12.0
                    )
                else:
                    nc.vector.reduce_max(
                        negmax[:], s_ps, axis=AX.X, negate=True
                    )
                nmx_prev["t"] = negmax
                u = wp.tile([RSEQ, S], sdt, name="u", tag="u")
                rs = wp.tile([RSEQ, 1], f32, name="rs", tag="rs")
                nc.scalar.activation(
                    u[:], s_ps, AF.Exp, bias=negmax[:], scale=1.0,
                    accum_out=rs[:],
                )
                rinv = wp.tile([RSEQ, 1], f32, name="rinv", tag="rinv")
                nc.vector.reciprocal(rinv[:], rs[:])
                gp = wp.tile([RSEQ, RSEQ], sdt, name="gp", tag="gp")
                nc.scalar.mul(gp[:], g_sb[:], rinv[:])
                dp = wp.tile([RSEQ, RSEQ], sdt, name="dp", tag="dp")
                nc.scalar.mul(dp[:], i8[:], rinv[:])
                nc.tensor.matmul(
                    wsum_ps, dp[:], u[:],
                    start=False, stop=(t == R - 1), skip_group_check=True,
                )
                if t < R - 1:
                    nc.tensor.matmul(
                        s_ps, gp[:], u[:],
                        start=False, stop=(t == R - 2), skip_group_check=True,
                    )

            def emit_wsum_bf():
                nc.vector.tensor_scalar_mul(wsum_bf[:], wsum_ps, 1.0)

            def emit_corr():
                """bt_c[k] = bT[k] + A^T[k] @ wsum  (bf16 out)."""
                for k0 in range(0, KT, 3):
                    ps = pstmp.tile([128, NFREE], f32, name="tmp", tag="tmp")
                    for kk in range(3):
                        k = k0 + kk
                        nc.tensor.matmul(
                            ps[:, kk * S:(kk + 1) * S],
                            a_bf[:, k * 128:(k + 1) * 128], wsum_bf[:],
                            start=(kk == 0), stop=(kk == 2),
                            skip_group_check=True,
                        )
                    for kk in range(3):
                        k = k0 + kk
                        nc.vector.tensor_tensor(
                            bt_c[k][:], bT_f[k][:], ps[:, kk * S:(kk + 1) * S],
                            op=ALU.add,
                        )

            def fin_m(m):
                """H chains for m on corrected b + fold into V-prod state."""
                ps = pstmp.tile([128, NFREE], f32, name="tmp", tag="tmp")
                msl = slice(m * 128, (m + 1) * 128)
                for k in range(KT):
                    nc.tensor.matmul(
                        ps[:, :S], pw_h[:, k * H3:(k + 1) * H3][:, msl],
                        bt_c[k][:],
                        start=(k == 0), stop=False, skip_group_check=True,
                    )
                for k in range(KT):
                    nc.tensor.matmul(
                        ps[:, S:2 * S], pw_t[:, k * H3:(k + 1) * H3][:, msl],
                        bt_c[k][:],
                        start=False, stop=(k == KT - 1),
                        skip_group_check=True,
                    )
                if use_pb:
                    nc.scalar.activation(
                        ps[:, :S], ps[:, :S], AF.Identity,
                        bias=pb_sb[:, m:m + 1], scale=1.0,
                    )
                if m < N_DVE:
                    nc.scalar.mul(hhn_bf[:, m * S:(m + 1) * S], ps[:, :S], -1.0)
                    vjap = ht4[m][:].rearrange("p (j r) -> p j r", r=4)
                    in0 = ps[:, S:2 * S].unsqueeze(2).broadcast_to([128, S, 4])
                    nc.scalar.copy(vjap, in0)
                else:
                    a = m - N_DVE
                    asl = slice(a * S, (a + 1) * S)
                    nc.scalar.copy(hh_fA[:, asl], ps[:, :S])
                    nc.scalar.copy(ht_bA[:, asl], ps[:, S:2 * S])

            def pair_thunk(m, sup, fin_after):
                def go():
                    vt = vd_t[m][sup % NRING]
                    i0 = sup * SUPI
                    in0 = (
                        ht4[m][:].rearrange("p (j r) -> p j r", r=4)
                        .unsqueeze(1).broadcast_to([128, WIDE, S, 4])
                    )
                    hq = hhn_bf[:, m * S + i0: m * S + i0 + SUPI].rearrange(
                        "p (g i) -> p g i", g=WIDE
                    )
                    in1 = hq.unsqueeze(2).broadcast_to([128, WIDE, S, 4])
                    vv = vt[:].rearrange(
                        "p (g j i) -> p g j i", g=WIDE, i=IGRP
                    )
                    nc.vector.tensor_tensor(vv, in0, in1, op=ALU.max)
                    if fin_after:
                        fin_m(m)
                return go

            def act_thunk(a, sup, gl, fin_after):
                def go():
                    vt = va_t[a][sup % NRING]
                    for ii in range(IGRP):
                        i = sup * SUPI + gl * IGRP + ii
                        dst = vt[:, gl * NFREE:(gl + 1) * NFREE].rearrange(
                            "p (j r) -> p j r", r=4
                        )[:, :, ii]
                        nc.scalar.activation(
                            dst, ht_bA[:, a * S:(a + 1) * S], AF.Relu,
                            bias=hh_fA[:, a * S + i: a * S + i + 1], scale=1.0,
                        )
                    if fin_after:
                        fin_m(N_DVE + a)
                return go

            def make_prod_units(sup, with_fin):
                units = []
                for m in DVE_MS:
                    units.append(pair_thunk(m, sup, with_fin))
                for a in range(N_ACT):
                    for gl in range(WIDE):
                        units.append(
                            act_thunk(a, sup, gl, with_fin and gl == WIDE - 1)
                        )
                return units

            pend = []

            def chain(g):
                sup, gl = g // WIDE, g % WIDE
                ops = pso.tile([C, NFREE], f32, name="ops", tag="ops")
                mlist = DVE_MS + ACT_MS
                for j, m in enumerate(mlist):
                    vt = (vd_t[m][sup % NRING] if m < N_DVE
                          else va_t[m - N_DVE][sup % NRING])
                    nc.tensor.matmul(
                        ops[:], rwr(m), vt[:, gl * NFREE:(gl + 1) * NFREE],
                        start=(j == 0), stop=False,
                    )
                # + P[c, i] broadcast over j via constant selector
                base = (g // 8) * 32
                q = g % 8
                nc.tensor.matmul(
                    ops[:], pT_sb[base:base + 32, :],
                    sel_sb[base:base + 32, q * NFREE:(q + 1) * NFREE],
                    start=False, stop=True,
                )
                pend.append((g, ops))

            def flush_one():
                g0, ops0 = pend.pop(0)
                ostg = ostg_t[g0 % 4]
                src = ops0[:].rearrange("c (j i) -> c i j", i=IGRP)
                dst = ostg[:].rearrange("c (i j) -> c i j", j=S)
                nc.scalar.copy(dst, src)
                nc.sync.dma_start(
                    out[:, g0 * NFREE:(g0 + 1) * NFREE], ostg[:]
                )

            def emit_p():
                # pT[i, c] = -(hhn^T rw)[i, c] = (rw^T hh)^T, DVE-owned m only
                ps = pstmp.tile([128, NFREE], f32, name="tmp", tag="tmp")
                for j, m in enumerate(DVE_MS):
                    nc.tensor.matmul(
                        ps[:S, :C], hhn_bf[:, m * S:(m + 1) * S], rwr(m),
                        start=(j == 0), stop=(j == N_DVE - 1),
                        skip_group_check=True,
                    )
                nc.vector.tensor_scalar_mul(pT_sb[:], ps[:S, :C], -1.0)

            def emit_from(q, n):
                for _ in range(n):
                    if q:
                        q.pop(0)()

            def emit_body(do_scan, prod_next_sup0):
                # super w produced in slots [max(0, 2w-4) .. 2w-1]; ring-3
                # parity guarantees writes land after the last chain read of
                # super w-3 (slots 2w-6, 2w-5).
                prod_qs = {
                    w: make_prod_units(w, with_fin=(do_scan and w == NSUP - 1))
                    for w in range(1, NSUP)
                }
                windows = {
                    w: list(range(3 * (w - 1), 3 * w))
                    for w in range(1, NSUP)
                }
                quota = {}
                for w, sl in windows.items():
                    n = len(prod_qs[w])
                    base = n // len(sl)
                    ext = n - base * len(sl)
                    for idx, s_ in enumerate(sl):
                        quota[(w, s_)] = base + (1 if idx < ext else 0)
                sup0_q = make_prod_units(0, False) if prod_next_sup0 else []
                step_i = {"n": 0}
                for slot in range(NSLOT):
                    if do_scan and slot == 12:
                        emit_wsum_bf()
                        emit_corr()
                    if do_scan and slot == 13:
                        phase_a()
                    halves = ([], [])
                    for w in range(1, NSUP):
                        q = quota.get((w, slot))
                        if q:
                            halves[0].append((prod_qs[w], (q + 1) // 2))
                            halves[1].append((prod_qs[w], q // 2))
                    if do_scan and slot < 12:
                        scan_step(step_i["n"])
                        step_i["n"] += 1
                    for qq, n in halves[0]:
                        emit_from(qq, n)
                    if do_scan and slot < 12:
                        scan_step(step_i["n"])
                        step_i["n"] += 1
                    for qq, n in halves[1]:
                        emit_from(qq, n)
                    if prod_next_sup0 and slot >= 21:
                        emit_from(sup0_q, 7)
                    chain(slot)
                    if len(pend) > 2:
                        flush_one()
                for w in range(1, NSUP):
                    emit_from(prod_qs[w], 99)
                if prod_next_sup0:
                    emit_from(sup0_q, 99)
                while pend:
                    flush_one()
                if do_scan:
                    emit_p()

            def emit_prologue():
                phase_a()
                for t in range(R):
                    scan_step(t)
                emit_wsum_bf()
                emit_corr()
                for m in range(MT):
                    fin_m(m)
                emit_p()
                phase_a()
                for th in make_prod_units(0, False):
                    th()

            emit_prologue()
            n_bodies = repeat - 1
            if n_bodies >= 4:
                with tc.For_i(0, n_bodies // 4, 1):
                    for _ in range(4):
                        emit_body(do_scan=True, prod_next_sup0=True)
            for _ in range(n_bodies % 4):
                emit_body(do_scan=True, prod_next_sup0=True)
            emit_body(do_scan=False, prod_next_sup0=False)

    nc.finalize()
    return nc


_CACHED = {}


def _prep_in_maps(encoded_text, rel_types_encoded, proj_W, proj_b, rel_W):
    import ml_dtypes

    relw_perm = np.ascontiguousarray(
        np.asarray(rel_W, np.float32).reshape(H3, R, TAG)
        .transpose(0, 2, 1).reshape(H3, C)
    )
    rw_pack = np.zeros((128, MT * C), np.float32)
    for m in range(MT):
        rw_pack[:, m * C:(m + 1) * C] = relw_perm[m * 128:(m + 1) * 128, :]
    rw_pack = rw_pack.astype(ml_dtypes.bfloat16)

    pw = np.asarray(proj_W, np.float32)
    pw_pack = np.zeros((128, 2 * KT * H3), np.float32)
    for kt in range(2 * KT):
        pw_pack[:, kt * H3:(kt + 1) * H3] = pw[kt * 128:(kt + 1) * 128, :]
    pw_pack = pw_pack.astype(ml_dtypes.bfloat16)

    sel = np.zeros((96, 8 * NFREE), np.float32)
    for base in (0, 32, 64):
        for q in range(8):
            for j in range(S):
                for ii in range(IGRP):
                    sel[base + q * IGRP + ii,
                        q * NFREE + j * IGRP + ii] = 1.0
    sel = sel.astype(ml_dtypes.bfloat16)

    in_maps = []
    for i in range(B):
        in_maps.append({
            "enc": np.ascontiguousarray(encoded_text[i], dtype=np.float32),
            "arel": np.ascontiguousarray(
                rel_types_encoded[i], dtype=np.float32
            ),
            "pwbf": pw_pack,
            "projb": np.ascontiguousarray(proj_b, dtype=np.float32),
            "rwbf": rw_pack,
            "sel4": sel,
        })
    return in_maps


def _assemble(results, rel_b):
    outs = []
    for i in range(B):
        o = results[i]["out"].reshape(TAG, R, S, S)
        outs.append(o)
    full = np.stack(outs, axis=0).astype(np.float32)
    if np.any(rel_b):
        relb_perm = np.asarray(rel_b, dtype=np.float32).reshape(R, TAG).T
        full = full + relb_perm[None, :, :, None, None]
    return full


def kernel(encoded_text, rel_types_encoded, proj_W, proj_b, rel_W, rel_b):
    from concourse.bass_utils import run_bass_kernel_spmd

    use_pb = bool(np.any(np.asarray(proj_b)))
    key = (1, use_pb)
    if key not in _CACHED:
        _CACHED[key] = build_nc(repeat=1, use_pb=use_pb)
    in_maps = _prep_in_maps(
        encoded_text, rel_types_encoded, proj_W, proj_b, rel_W
    )
    res = run_bass_kernel_spmd(_CACHED[key], in_maps, list(range(B)))
    return _assemble(res.results, rel_b)


# revision 4
# speedup vs baseline: 1.0497x; 1.0041x over previous
"""Trainium2 Bass kernel v8 for nn_AttModel (B=8, S=96, D=768, R=24, RSEQ=8, TAG=3).

Data-parallel over batch: core i handles sample i.

v7 (on top of v6's pipelined body + max/P factorization):
  - relu(a+b) = max(a,-b)+b with the linear +hh term folded into the
    output as P = rw^T hh (tiny GEMM), added via the flush bias port.
  - The rank-8 refinement is applied to the TRANSPOSED encoder before
    the H GEMM: bT_corr[k] = bT[k] + A^T[k] @ wsum, so H = b_corr @ W
    comes out final -- no AW GEMM, no Delta MMs, no H0 staging buffers.
  - fin_m (slots 12-19, attached after m's last old-state read): hh/ht
    chains (12 MMs) into one PSUM bank, then stt -> hhn (DVE tiles) and
    jap TT -> HT4, or TT -> hh_fA/ht_bA (ACT tiles).
  - phase_a(t+1) (enc DMA, transposes, G, s0) runs in slots 13-16 so the
    next body's scan starts immediately.
  - scan: 2 steps/slot in slots 0-11; u/gp/dp in bf16 (full-rate PE MMs).
"""
import sys

sys.path.insert(0, "/opt/trn_rl_repo")

import numpy as np

S, D, H3 = 96, 768, 2304
R, RSEQ, TAG, C = 24, 8, 3, 72
B = 8
KT = D // 128
MT = H3 // 128
IGRP = 4
NG = S // IGRP           # 24
NFREE = IGRP * S         # 384
SCALE = 1.0 / float(np.sqrt(np.float32(D)))

N_DVE = 17
DVE_MS = list(range(N_DVE))
ACT_MS = list(range(N_DVE, MT))
N_ACT = len(ACT_MS)
WIDE = 4
NSUP = NG // WIDE        # 6
SUPI = WIDE * IGRP       # 16
NRING = 2                # v-tile ring depth (parity sup % 2)
NSLOT = NG


def build_nc(repeat: int = 1, use_pb: bool = False, scan_bf16: bool = True):
    import concourse.bass as bass
    from concourse import bacc, mybir
    import concourse.tile as tile
    from concourse.masks import make_identity

    f32 = mybir.dt.float32
    bf16 = mybir.dt.bfloat16
    AF = mybir.ActivationFunctionType
    ALU = mybir.AluOpType
    AX = mybir.AxisListType
    sdt = bf16 if scan_bf16 else f32

    nc = bacc.Bacc()
    enc = nc.dram_tensor("enc", [S, D], f32, kind="ExternalInput")
    arel = nc.dram_tensor("arel", [RSEQ, D], f32, kind="ExternalInput")
    pwbf = nc.dram_tensor("pwbf", [128, 2 * KT * H3], bf16, kind="ExternalInput")
    projb = nc.dram_tensor("projb", [H3], f32, kind="ExternalInput")
    rwbf = nc.dram_tensor("rwbf", [128, MT * C], bf16, kind="ExternalInput")
    sel4 = nc.dram_tensor("sel4", [96, 8 * NFREE], bf16, kind="ExternalInput")
    out = nc.dram_tensor("out", [C, S * S], f32, kind="ExternalOutput")

    with tile.TileContext(nc) as tc:
        with (
            tc.tile_pool(name="persist", bufs=1) as pp,
            tc.tile_pool(name="work", bufs=10) as wp,
            tc.tile_pool(name="pso", bufs=4, space="PSUM") as pso,
            tc.tile_pool(name="psss", bufs=1, space="PSUM") as psss,
            tc.tile_pool(name="pstmp", bufs=3, space="PSUM") as pstmp,
        ):
            # ---- once-only loads & constants ----
            ident = pp.tile([128, 128], f32, name="ident", tag="ident")
            make_identity(nc, ident[:])
            i8 = pp.tile([RSEQ, RSEQ], f32, name="i8", tag="i8")
            make_identity(nc, i8[:])
            rw_all = pp.tile([128, MT * C], bf16, name="rw_all", tag="rw_all")
            nc.sync.dma_start(rw_all[:], rwbf[:])
            pw_h = pp.tile([128, KT * H3], bf16, name="pw_h", tag="pw_h")
            nc.sync.dma_start(pw_h[:], pwbf[:, : KT * H3])
            pw_t = pp.tile([128, KT * H3], bf16, name="pw_t", tag="pw_t")
            nc.sync.dma_start(pw_t[:], pwbf[:, KT * H3:])
            pb_sb = pp.tile([128, MT], f32, name="pb", tag="pb")
            sel_sb = pp.tile([96, 8 * NFREE], bf16, name="sel_sb", tag="sel_sb")
            nc.sync.dma_start(sel_sb[:], sel4[:])
            if use_pb:
                nc.sync.dma_start(
                    pb_sb[:], projb.rearrange("(t p) -> p t", p=128)
                )

            def rwr(m):
                return rw_all[:, m * C:(m + 1) * C]

            # ---- persistent per-iteration state ----
            enc_nat = pp.tile([S, D], f32, name="enc_nat", tag="enc_nat")
            a_nat = pp.tile([RSEQ, D], f32, name="a_nat", tag="a_nat")
            a_bf = pp.tile([RSEQ, D], bf16, name="a_bf", tag="a_bf")
            bT_f = [pp.tile([128, S], f32, name=f"bTf{k}", tag=f"bTf{k}")
                    for k in range(KT)]
            at_raw = [pp.tile([128, RSEQ], f32, name=f"atr{k}", tag=f"atr{k}")
                      for k in range(KT)]
            at_scl = [pp.tile([128, RSEQ], f32, name=f"ats{k}", tag=f"ats{k}")
                      for k in range(KT)]
            g_sb = pp.tile([RSEQ, RSEQ], f32, name="g", tag="g")
            bt_c = [pp.tile([128, S], bf16, name=f"btc{k}", tag=f"btc{k}")
                    for k in range(KT)]
            wsum_bf = pp.tile([RSEQ, S], bf16, name="wsum_bf", tag="wsum_bf")
            hhn_bf = pp.tile([128, N_DVE * S], bf16, name="hhn", tag="hhn")
            ht4 = [pp.tile([128, NFREE], bf16, name=f"ht4_{m}", tag=f"ht4_{m}")
                   for m in DVE_MS]
            hh_fA = pp.tile([128, N_ACT * S], f32, name="hh_fA", tag="hh_fA")
            ht_bA = pp.tile([128, N_ACT * S], bf16, name="ht_bA", tag="ht_bA")
            vd_t = [
                [pp.tile([128, WIDE * NFREE], bf16, name=f"vd{m}_{p}",
                         tag=f"vd{m}_{p}") for p in range(NRING)]
                for m in DVE_MS
            ]
            va_t = [
                [pp.tile([128, WIDE * NFREE], bf16, name=f"va{a}_{p}",
                         tag=f"va{a}_{p}") for p in range(NRING)]
                for a in range(N_ACT)
            ]
            ostg_t = [pp.tile([C, NFREE], f32, name=f"ostg{r_}", tag=f"ostg{r_}")
                      for r_ in range(3)]
            pT_sb = pp.tile([S, C], bf16, name="pT_sb", tag="pT_sb")

            # s and wsum share one PSUM bank: the s0 chain's start=True
            # zeroes the whole bank, so all wsum MMs accumulate start=False.
            sw_full = psss.tile([RSEQ, 2 * S], f32, name="sw", tag="sw", bufs=1)
            s_ps = sw_full[:, :S]
            wsum_ps = sw_full[:, S:]

            # ---- emission helpers ----
            def phase_a():
                """Loads + transposes + G + s0 for the NEXT scan."""
                nc.sync.dma_start(enc_nat[:], enc[:])
                nc.sync.dma_start(a_nat[:], arel[:])
                nc.vector.tensor_scalar_mul(a_bf[:], a_nat[:], 1.0)
                for k in range(KT):
                    ps = pstmp.tile([128, NFREE], f32, name="tmp", tag="tmp")
                    nc.tensor.transpose(
                        ps[:, :S], enc_nat[:, k * 128:(k + 1) * 128],
                        ident[:S, :S],
                    )
                    nc.scalar.copy(bT_f[k][:], ps[:, :S])
                for k in range(KT):
                    ps = pstmp.tile([128, NFREE], f32, name="tmp", tag="tmp")
                    nc.tensor.transpose(
                        ps[:, :RSEQ], a_nat[:, k * 128:(k + 1) * 128],
                        ident[:RSEQ, :RSEQ],
                    )
                    nc.scalar.copy(at_raw[k][:], ps[:, :RSEQ])
                    nc.scalar.mul(at_scl[k][:], ps[:, :RSEQ], SCALE)
                gtmp = pstmp.tile([128, NFREE], f32, name="tmp", tag="tmp")
                for k in range(KT):
                    nc.tensor.matmul(
                        gtmp[:RSEQ, :RSEQ], at_scl[k][:], at_raw[k][:],
                        start=(k == 0), stop=(k == KT - 1),
                        skip_group_check=True,
                    )
                nc.vector.tensor_scalar_mul(g_sb[:], gtmp[:RSEQ, :RSEQ], 1.0)
                for k in range(KT):
                    nc.tensor.matmul(
                        s_ps, at_scl[k][:], bT_f[k][:],
                        start=(k == 0), stop=False, skip_group_check=True,
                    )

            nmx_prev = {"t": None}

            def scan_step(t):
                negmax = wp.tile([RSEQ, 1], f32, name="negmax", tag="negmax")
                if t % 4 != 0 and nmx_prev["t"] is not None:
                    nc.vector.tensor_scalar_add(
                        negmax[:], nmx_prev["t"][:], -12.0
                    )
                else:
                    nc.vector.reduce_max(
                        negmax[:], s_ps, axis=AX.X, negate=True
                    )
                nmx_prev["t"] = negmax
                u = wp.tile([RSEQ, S], sdt, name="u", tag="u")
                rs = wp.tile([RSEQ, 1], f32, name="rs", tag="rs")
                nc.scalar.activation(
                    u[:], s_ps, AF.Exp, bias=negmax[:], scale=1.0,
                    accum_out=rs[:],
                )
                rinv = wp.tile([RSEQ, 1], f32, name="rinv", tag="rinv")
                nc.vector.reciprocal(rinv[:], rs[:])
                gp = wp.tile([RSEQ, RSEQ], sdt, name="gp", tag="gp")
                nc.scalar.mul(gp[:], g_sb[:], rinv[:])
                dp = wp.tile([RSEQ, RSEQ], sdt, name="dp", tag="dp")
                nc.scalar.mul(dp[:], i8[:], rinv[:])
                nc.tensor.matmul(
                    wsum_ps, dp[:], u[:],
                    start=False, stop=(t == R - 1), skip_group_check=True,
                )
                if t < R - 1:
                    nc.tensor.matmul(
                        s_ps, gp[:], u[:],
                        start=False, stop=(t == R - 2), skip_group_check=True,
                    )

            def emit_wsum_bf():
                nc.vector.tensor_scalar_mul(wsum_bf[:], wsum_ps, 1.0)

            def emit_corr():
                """bt_c[k] = bT[k] + A^T[k] @ wsum  (bf16 out)."""
                for k0 in range(0, KT, 3):
                    ps = pstmp.tile([128, NFREE], f32, name="tmp", tag="tmp")
                    for kk in range(3):
                        k = k0 + kk
                        nc.tensor.matmul(
                            ps[:, kk * S:(kk + 1) * S],
                            a_bf[:, k * 128:(k + 1) * 128], wsum_bf[:],
                            start=(kk == 0), stop=(kk == 2),
                            skip_group_check=True,
                        )
                    for kk in range(3):
                        k = k0 + kk
                        nc.vector.tensor_tensor(
                            bt_c[k][:], bT_f[k][:], ps[:, kk * S:(kk + 1) * S],
                            op=ALU.add,
                        )

            def fin_m(m):
                """H chains for m on corrected b + fold into V-prod state."""
                ps = pstmp.tile([128, NFREE], f32, name="tmp", tag="tmp")
                msl = slice(m * 128, (m + 1) * 128)
                for k in range(KT):
                    nc.tensor.matmul(
                        ps[:, :S], pw_h[:, k * H3:(k + 1) * H3][:, msl],
                        bt_c[k][:],
                        start=(k == 0), stop=False, skip_group_check=True,
                    )
                for k in range(KT):
                    nc.tensor.matmul(
                        ps[:, S:2 * S], pw_t[:, k * H3:(k + 1) * H3][:, msl],
                        bt_c[k][:],
                        start=False, stop=(k == KT - 1),
                        skip_group_check=True,
                    )
                if use_pb:
                    nc.scalar.activation(
                        ps[:, :S], ps[:, :S], AF.Identity,
                        bias=pb_sb[:, m:m + 1], scale=1.0,
                    )
                if m < N_DVE:
                    nc.scalar.mul(hhn_bf[:, m * S:(m + 1) * S], ps[:, :S], -1.0)
                    vjap = ht4[m][:].rearrange("p (j r) -> p j r", r=4)
                    in0 = ps[:, S:2 * S].unsqueeze(2).broadcast_to([128, S, 4])
                    nc.scalar.copy(vjap, in0)
                else:
                    a = m - N_DVE
                    asl = slice(a * S, (a + 1) * S)
                    nc.scalar.copy(hh_fA[:, asl], ps[:, :S])
                    nc.scalar.copy(ht_bA[:, asl], ps[:, S:2 * S])

            def pair_thunk(m, sup, fin_after):
                def go():
                    vt = vd_t[m][sup % NRING]
                    i0 = sup * SUPI
                    in0 = (
                        ht4[m][:].rearrange("p (j r) -> p j r", r=4)
                        .unsqueeze(1).broadcast_to([128, WIDE, S, 4])
                    )
                    hq = hhn_bf[:, m * S + i0: m * S + i0 + SUPI].rearrange(
                        "p (g i) -> p g i", g=WIDE
                    )
                    in1 = hq.unsqueeze(2).broadcast_to([128, WIDE, S, 4])
                    vv = vt[:].rearrange(
                        "p (g j i) -> p g j i", g=WIDE, i=IGRP
                    )
                    nc.vector.tensor_tensor(vv, in0, in1, op=ALU.max)
                    if fin_after:
                        fin_m(m)
                return go

            def act_thunk(a, sup, gl, fin_after):
                def go():
                    vt = va_t[a][sup % NRING]
                    for ii in range(IGRP):
                        i = sup * SUPI + gl * IGRP + ii
                        dst = vt[:, gl * NFREE:(gl + 1) * NFREE].rearrange(
                            "p (j r) -> p j r", r=4
                        )[:, :, ii]
                        nc.scalar.activation(
                            dst, ht_bA[:, a * S:(a + 1) * S], AF.Relu,
                            bias=hh_fA[:, a * S + i: a * S + i + 1], scale=1.0,
                        )
                    if fin_after:
                        fin_m(N_DVE + a)
                return go

            def make_prod_units(sup, with_fin):
                units = []
                for m in DVE_MS:
                    units.append(pair_thunk(m, sup, with_fin))
                for a in range(N_ACT):
                    for gl in range(WIDE):
                        units.append(
                            act_thunk(a, sup, gl, with_fin and gl == WIDE - 1)
                        )
                return units

            pend = []

            def chain(g):
                sup, gl = g // WIDE, g % WIDE
                ops = pso.tile([C, NFREE], f32, name="ops", tag="ops")
                mlist = DVE_MS + ACT_MS
                for j, m in enumerate(mlist):
                    vt = (vd_t[m][sup % NRING] if m < N_DVE
                          else va_t[m - N_DVE][sup % NRING])
                    nc.tensor.matmul(
                        ops[:], rwr(m), vt[:, gl * NFREE:(gl + 1) * NFREE],
                        start=(j == 0), stop=False,
                    )
                # + P[c, i] broadcast over j via constant selector
                base = (g // 8) * 32
                q = g % 8
                nc.tensor.matmul(
                    ops[:], pT_sb[base:base + 32, :],
                    sel_sb[base:base + 32, q * NFREE:(q + 1) * NFREE],
                    start=False, stop=True,
                )
                pend.append((g, ops))

            def flush_one():
                g0, ops0 = pend.pop(0)
                ostg = ostg_t[g0 % 3]
                src = ops0[:].rearrange("c (j i) -> c i j", i=IGRP)
                dst = ostg[:].rearrange("c (i j) -> c i j", j=S)
                nc.scalar.copy(dst, src)
                nc.sync.dma_start(
                    out[:, g0 * NFREE:(g0 + 1) * NFREE], ostg[:]
                )

            def emit_p():
                # pT[i, c] = -(hhn^T rw)[i, c] = (rw^T hh)^T, DVE-owned m only
                ps = pstmp.tile([128, NFREE], f32, name="tmp", tag="tmp")
                for j, m in enumerate(DVE_MS):
                    nc.tensor.matmul(
                        ps[:S, :C], hhn_bf[:, m * S:(m + 1) * S], rwr(m),
                        start=(j == 0), stop=(j == N_DVE - 1),
                        skip_group_check=True,
                    )
                nc.vector.tensor_scalar_mul(pT_sb[:], ps[:S, :C], -1.0)

            def emit_from(q, n):
                for _ in range(n):
                    if q:
                        q.pop(0)()

            def emit_body(do_scan, prod_next_sup0):
                # super w produced in slots [max(0, 2w-4) .. 2w-1]; ring-3
                # parity guarantees writes land after the last chain read of
                # super w-3 (slots 2w-6, 2w-5).
                prod_qs = {
                    w: make_prod_units(w, with_fin=(do_scan and w == NSUP - 1))
                    for w in range(1, NSUP)
                }
                windows = {
                    w: list(range(WIDE * (w - 1), WIDE * w))
                    for w in range(1, NSUP)
                }
                quota = {}
                for w, sl in windows.items():
                    n = len(prod_qs[w])
                    base = n // len(sl)
                    ext = n - base * len(sl)
                    for idx, s_ in enumerate(sl):
                        quota[(w, s_)] = base + (1 if idx < ext else 0)
                sup0_q = make_prod_units(0, False) if prod_next_sup0 else []
                step_i = {"n": 0}
                for slot in range(NSLOT):
                    if do_scan and slot == 12:
                        emit_wsum_bf()
                        emit_corr()
                    if do_scan and slot == 13:
                        phase_a()
                    halves = ([], [])
                    for w in range(1, NSUP):
                        q = quota.get((w, slot))
                        if q:
                            halves[0].append((prod_qs[w], (q + 1) // 2))
                            halves[1].append((prod_qs[w], q // 2))
                    if do_scan and slot < 12:
                        scan_step(step_i["n"])
                        step_i["n"] += 1
                    for qq, n in halves[0]:
                        emit_from(qq, n)
                    if do_scan and slot < 12:
                        scan_step(step_i["n"])
                        step_i["n"] += 1
                    for qq, n in halves[1]:
                        emit_from(qq, n)
                    if prod_next_sup0 and slot >= NSLOT - WIDE:
                        emit_from(sup0_q, 6)
                    chain(slot)
                    if len(pend) > 2:
                        flush_one()
                for w in range(1, NSUP):
                    emit_from(prod_qs[w], 99)
                if prod_next_sup0:
                    emit_from(sup0_q, 99)
                while pend:
                    flush_one()
                if do_scan:
                    emit_p()

            def emit_prologue():
                phase_a()
                for t in range(R):
                    scan_step(t)
                emit_wsum_bf()
                emit_corr()
                for m in range(MT):
                    fin_m(m)
                emit_p()
                phase_a()
                for th in make_prod_units(0, False):
                    th()

            emit_prologue()
            n_bodies = repeat - 1
            if n_bodies >= 4:
                with tc.For_i(0, n_bodies // 4, 1):
                    for _ in range(4):
                        emit_body(do_scan=True, prod_next_sup0=True)
            for _ in range(n_bodies % 4):
                emit_body(do_scan=True, prod_next_sup0=True)
            emit_body(do_scan=False, prod_next_sup0=False)

    nc.finalize()
    return nc


_CACHED = {}


def _prep_in_maps(encoded_text, rel_types_encoded, proj_W, proj_b, rel_W):
    import ml_dtypes

    relw_perm = np.ascontiguousarray(
        np.asarray(rel_W, np.float32).reshape(H3, R, TAG)
        .transpose(0, 2, 1).reshape(H3, C)
    )
    rw_pack = np.zeros((128, MT * C), np.float32)
    for m in range(MT):
        rw_pack[:, m * C:(m + 1) * C] = relw_perm[m * 128:(m + 1) * 128, :]
    rw_pack = rw_pack.astype(ml_dtypes.bfloat16)

    pw = np.asarray(proj_W, np.float32)
    pw_pack = np.zeros((128, 2 * KT * H3), np.float32)
    for kt in range(2 * KT):
        pw_pack[:, kt * H3:(kt + 1) * H3] = pw[kt * 128:(kt + 1) * 128, :]
    pw_pack = pw_pack.astype(ml_dtypes.bfloat16)

    sel = np.zeros((96, 8 * NFREE), np.float32)
    for base in (0, 32, 64):
        for q in range(8):
            for j in range(S):
                for ii in range(IGRP):
                    sel[base + q * IGRP + ii,
                        q * NFREE + j * IGRP + ii] = 1.0
    sel = sel.astype(ml_dtypes.bfloat16)

    in_maps = []
    for i in range(B):
        in_maps.append({
            "enc": np.ascontiguousarray(encoded_text[i], dtype=np.float32),
            "arel": np.ascontiguousarray(
                rel_types_encoded[i], dtype=np.float32
            ),
            "pwbf": pw_pack,
            "projb": np.ascontiguousarray(proj_b, dtype=np.float32),
            "rwbf": rw_pack,
            "sel4": sel,
        })
    return in_maps


def _assemble(results, rel_b):
    outs = []
    for i in range(B):
        o = results[i]["out"].reshape(TAG, R, S, S)
        outs.append(o)
    full = np.stack(outs, axis=0).astype(np.float32)
    if np.any(rel_b):
        relb_perm = np.asarray(rel_b, dtype=np.float32).reshape(R, TAG).T
        full = full + relb_perm[None, :, :, None, None]
    return full


def kernel(encoded_text, rel_types_encoded, proj_W, proj_b, rel_W, rel_b):
    from concourse.bass_utils import run_bass_kernel_spmd

    use_pb = bool(np.any(np.asarray(proj_b)))
    key = (1, use_pb)
    if key not in _CACHED:
        _CACHED[key] = build_nc(repeat=1, use_pb=use_pb)
    in_maps = _prep_in_maps(
        encoded_text, rel_types_encoded, proj_W, proj_b, rel_W
    )
    res = run_bass_kernel_spmd(_CACHED[key], in_maps, list(range(B)))
    return _assemble(res.results, rel_b)


# revision 6
# speedup vs baseline: 1.0629x; 1.0126x over previous
"""Trainium2 Bass kernel v8 for nn_AttModel (B=8, S=96, D=768, R=24, RSEQ=8, TAG=3).

Data-parallel over batch: core i handles sample i.

v7 (on top of v6's pipelined body + max/P factorization):
  - relu(a+b) = max(a,-b)+b with the linear +hh term folded into the
    output as P = rw^T hh (tiny GEMM), added via the flush bias port.
  - The rank-8 refinement is applied to the TRANSPOSED encoder before
    the H GEMM: bT_corr[k] = bT[k] + A^T[k] @ wsum, so H = b_corr @ W
    comes out final -- no AW GEMM, no Delta MMs, no H0 staging buffers.
  - fin_m (slots 12-19, attached after m's last old-state read): hh/ht
    chains (12 MMs) into one PSUM bank, then stt -> hhn (DVE tiles) and
    jap TT -> HT4, or TT -> hh_fA/ht_bA (ACT tiles).
  - phase_a(t+1) (enc DMA, transposes, G, s0) runs in slots 13-16 so the
    next body's scan starts immediately.
  - scan: 2 steps/slot in slots 0-11; u/gp/dp in bf16 (full-rate PE MMs).
"""
import sys

sys.path.insert(0, "/opt/trn_rl_repo")

import numpy as np

S, D, H3 = 96, 768, 2304
R, RSEQ, TAG, C = 24, 8, 3, 72
B = 8
KT = D // 128
MT = H3 // 128
IGRP = 4
NG = S // IGRP           # 24
NFREE = IGRP * S         # 384
SCALE = 1.0 / float(np.sqrt(np.float32(D)))

N_DVE = 17
DVE_MS = list(range(N_DVE))
ACT_MS = list(range(N_DVE, MT))
N_ACT = len(ACT_MS)
WIDE = 4
NSUP = NG // WIDE        # 6
SUPI = WIDE * IGRP       # 16
NRING = 2                # v-tile ring depth (parity sup % 2)
NSLOT = NG


def build_nc(repeat: int = 1, use_pb: bool = False, scan_bf16: bool = True):
    import concourse.bass as bass
    from concourse import bacc, mybir
    import concourse.tile as tile
    from concourse.masks import make_identity

    f32 = mybir.dt.float32
    bf16 = mybir.dt.bfloat16
    AF = mybir.ActivationFunctionType
    ALU = mybir.AluOpType
    AX = mybir.AxisListType
    sdt = bf16 if scan_bf16 else f32

    nc = bacc.Bacc()
    enc = nc.dram_tensor("enc", [S, D], f32, kind="ExternalInput")
    arel = nc.dram_tensor("arel", [RSEQ, D], f32, kind="ExternalInput")
    pwbf = nc.dram_tensor("pwbf", [128, 2 * KT * H3], bf16, kind="ExternalInput")
    projb = nc.dram_tensor("projb", [H3], f32, kind="ExternalInput")
    rwbf = nc.dram_tensor("rwbf", [128, MT * C], bf16, kind="ExternalInput")
    sel4 = nc.dram_tensor("sel4", [96, 8 * NFREE], bf16, kind="ExternalInput")
    out = nc.dram_tensor("out", [C, S * S], f32, kind="ExternalOutput")

    with tile.TileContext(nc) as tc:
        with (
            tc.tile_pool(name="persist", bufs=1) as pp,
            tc.tile_pool(name="work", bufs=10) as wp,
            tc.tile_pool(name="pso", bufs=4, space="PSUM") as pso,
            tc.tile_pool(name="psss", bufs=1, space="PSUM") as psss,
            tc.tile_pool(name="pstmp", bufs=3, space="PSUM") as pstmp,
        ):
            # ---- once-only loads & constants ----
            ident = pp.tile([128, 128], f32, name="ident", tag="ident")
            make_identity(nc, ident[:])
            i8 = pp.tile([RSEQ, RSEQ], f32, name="i8", tag="i8")
            make_identity(nc, i8[:])
            rw_all = pp.tile([128, MT * C], bf16, name="rw_all", tag="rw_all")
            nc.sync.dma_start(rw_all[:], rwbf[:])
            pw_h = pp.tile([128, KT * H3], bf16, name="pw_h", tag="pw_h")
            nc.sync.dma_start(pw_h[:], pwbf[:, : KT * H3])
            pw_t = pp.tile([128, KT * H3], bf16, name="pw_t", tag="pw_t")
            nc.sync.dma_start(pw_t[:], pwbf[:, KT * H3:])
            pb_sb = pp.tile([128, MT], f32, name="pb", tag="pb")
            sel_sb = pp.tile([96, 8 * NFREE], bf16, name="sel_sb", tag="sel_sb")
            nc.sync.dma_start(sel_sb[:], sel4[:])
            if use_pb:
                nc.sync.dma_start(
                    pb_sb[:], projb.rearrange("(t p) -> p t", p=128)
                )

            def rwr(m):
                return rw_all[:, m * C:(m + 1) * C]

            # ---- persistent per-iteration state ----
            enc_nat = pp.tile([S, D], f32, name="enc_nat", tag="enc_nat")
            a_nat = pp.tile([RSEQ, D], f32, name="a_nat", tag="a_nat")
            a_bf = pp.tile([RSEQ, D], bf16, name="a_bf", tag="a_bf")
            bT_f = [pp.tile([128, S], bf16, name=f"bTf{k}", tag=f"bTf{k}",
                            padded_shape=[128, 2 * S])
                    for k in range(KT)]
            at_raw = [pp.tile([128, RSEQ], bf16, name=f"atr{k}",
                              tag=f"atr{k}", padded_shape=[128, 2 * RSEQ])
                      for k in range(KT)]
            at_scl = [pp.tile([128, RSEQ], bf16, name=f"ats{k}",
                              tag=f"ats{k}", padded_shape=[128, 2 * RSEQ])
                      for k in range(KT)]
            g_sb = pp.tile([RSEQ, RSEQ], f32, name="g", tag="g")
            bt_c = [pp.tile([128, S], bf16, name=f"btc{k}", tag=f"btc{k}")
                    for k in range(KT)]
            wsum_bf = pp.tile([RSEQ, S], bf16, name="wsum_bf", tag="wsum_bf")
            hhn_bf = pp.tile([128, N_DVE * S], bf16, name="hhn", tag="hhn")
            ht4 = [pp.tile([128, NFREE], bf16, name=f"ht4_{m}", tag=f"ht4_{m}")
                   for m in DVE_MS]
            hh_fA = pp.tile([128, N_ACT * S], f32, name="hh_fA", tag="hh_fA")
            ht_bA = pp.tile([128, N_ACT * S], bf16, name="ht_bA", tag="ht_bA")
            vd_t = [
                [pp.tile([128, WIDE * NFREE], bf16, name=f"vd{m}_{p}",
                         tag=f"vd{m}_{p}") for p in range(NRING)]
                for m in DVE_MS
            ]
            va_t = [
                [pp.tile([128, WIDE * NFREE], bf16, name=f"va{a}_{p}",
                         tag=f"va{a}_{p}") for p in range(NRING)]
                for a in range(N_ACT)
            ]
            ostg_t = [pp.tile([C, NFREE], f32, name=f"ostg{r_}", tag=f"ostg{r_}")
                      for r_ in range(3)]
            pT_sb = pp.tile([S, C], bf16, name="pT_sb", tag="pT_sb")

            # s and wsum share one PSUM bank: the s0 chain's start=True
            # zeroes the whole bank, so all wsum MMs accumulate start=False.
            sw_full = psss.tile([RSEQ, 2 * S], f32, name="sw", tag="sw", bufs=1)
            s_ps = sw_full[:, :S]
            wsum_ps = sw_full[:, S:]

            # ---- emission helpers ----
            def phase_a():
                """Loads + transposes + G + s0 for the NEXT scan."""
                nc.sync.dma_start(enc_nat[:], enc[:])
                nc.sync.dma_start(a_nat[:], arel[:])
                nc.scalar.copy(a_bf[:], a_nat[:])
                for k in range(KT):
                    ps = pstmp.tile([128, NFREE], f32, name="tmp", tag="tmp")
                    nc.tensor.transpose(
                        ps[:, :S], enc_nat[:, k * 128:(k + 1) * 128],
                        ident[:S, :S],
                    )
                    nc.scalar.copy(bT_f[k][:], ps[:, :S])
                for k in range(KT):
                    ps = pstmp.tile([128, NFREE], f32, name="tmp", tag="tmp")
                    nc.tensor.transpose(
                        ps[:, :RSEQ], a_nat[:, k * 128:(k + 1) * 128],
                        ident[:RSEQ, :RSEQ],
                    )
                    nc.scalar.copy(at_raw[k][:], ps[:, :RSEQ])
                    nc.scalar.mul(at_scl[k][:], ps[:, :RSEQ], SCALE)
                gtmp = pstmp.tile([128, NFREE], f32, name="tmp", tag="tmp")
                for k in range(KT):
                    nc.tensor.matmul(
                        gtmp[:RSEQ, :RSEQ], at_scl[k][:], at_raw[k][:],
                        start=(k == 0), stop=(k == KT - 1),
                        skip_group_check=True,
                    )
                nc.scalar.copy(g_sb[:], gtmp[:RSEQ, :RSEQ])
                for k in range(KT):
                    nc.tensor.matmul(
                        s_ps, at_scl[k][:], bT_f[k][:],
                        start=(k == 0), stop=False, skip_group_check=True,
                    )

            nmx_prev = {"t": None}

            def scan_step(t):
                negmax = wp.tile([RSEQ, 1], f32, name="negmax", tag="negmax")
                if t % 4 != 0 and nmx_prev["t"] is not None:
                    nc.vector.tensor_scalar_add(
                        negmax[:], nmx_prev["t"][:], -12.0
                    )
                else:
                    nc.vector.reduce_max(
                        negmax[:], s_ps, axis=AX.X, negate=True
                    )
                nmx_prev["t"] = negmax
                u = wp.tile([RSEQ, S], sdt, name="u", tag="u")
                rs = wp.tile([RSEQ, 1], f32, name="rs", tag="rs")
                nc.scalar.activation(
                    u[:], s_ps, AF.Exp, bias=negmax[:], scale=1.0,
                    accum_out=rs[:],
                )
                rinv = wp.tile([RSEQ, 1], f32, name="rinv", tag="rinv")
                nc.vector.reciprocal(rinv[:], rs[:])
                gp = wp.tile([RSEQ, RSEQ], sdt, name="gp", tag="gp")
                nc.scalar.mul(gp[:], g_sb[:], rinv[:])
                dp = wp.tile([RSEQ, RSEQ], sdt, name="dp", tag="dp")
                nc.scalar.mul(dp[:], i8[:], rinv[:])
                nc.tensor.matmul(
                    wsum_ps, dp[:], u[:],
                    start=False, stop=(t == R - 1), skip_group_check=True,
                )
                if t < R - 1:
                    nc.tensor.matmul(
                        s_ps, gp[:], u[:],
                        start=False, stop=(t == R - 2), skip_group_check=True,
                    )

            def emit_wsum_bf():
                nc.scalar.copy(wsum_bf[:], wsum_ps)

            def emit_corr():
                """bt_c[k] = bT[k] + A^T[k] @ wsum  (bf16 out)."""
                for k0 in range(0, KT, 3):
                    ps = pstmp.tile([128, NFREE], f32, name="tmp", tag="tmp")
                    for kk in range(3):
                        k = k0 + kk
                        nc.tensor.matmul(
                            ps[:, kk * S:(kk + 1) * S],
                            a_bf[:, k * 128:(k + 1) * 128], wsum_bf[:],
                            start=(kk == 0), stop=(kk == 2),
                            skip_group_check=True,
                        )
                    for kk in range(3):
                        k = k0 + kk
                        nc.vector.tensor_tensor(
                            bt_c[k][:], bT_f[k][:], ps[:, kk * S:(kk + 1) * S],
                            op=ALU.add,
                        )

            def fin_m(m):
                """H chains for m on corrected b + fold into V-prod state."""
                ps = pstmp.tile([128, NFREE], f32, name="tmp", tag="tmp")
                msl = slice(m * 128, (m + 1) * 128)
                for k in range(KT):
                    nc.tensor.matmul(
                        ps[:, :S], pw_h[:, k * H3:(k + 1) * H3][:, msl],
                        bt_c[k][:],
                        start=(k == 0), stop=False, skip_group_check=True,
                    )
                for k in range(KT):
                    nc.tensor.matmul(
                        ps[:, S:2 * S], pw_t[:, k * H3:(k + 1) * H3][:, msl],
                        bt_c[k][:],
                        start=False, stop=(k == KT - 1),
                        skip_group_check=True,
                    )
                if use_pb:
                    nc.scalar.activation(
                        ps[:, :S], ps[:, :S], AF.Identity,
                        bias=pb_sb[:, m:m + 1], scale=1.0,
                    )
                if m < N_DVE:
                    nc.scalar.mul(hhn_bf[:, m * S:(m + 1) * S], ps[:, :S], -1.0)
                    vjap = ht4[m][:].rearrange("p (j r) -> p j r", r=4)
                    in0 = ps[:, S:2 * S].unsqueeze(2).broadcast_to([128, S, 4])
                    nc.scalar.copy(vjap, in0)
                else:
                    a = m - N_DVE
                    asl = slice(a * S, (a + 1) * S)
                    nc.scalar.copy(hh_fA[:, asl], ps[:, :S])
                    nc.scalar.copy(ht_bA[:, asl], ps[:, S:2 * S])

            def pair_thunk(m, sup, fin_after):
                def go():
                    vt = vd_t[m][sup % NRING]
                    i0 = sup * SUPI
                    in0 = (
                        ht4[m][:].rearrange("p (j r) -> p j r", r=4)
                        .unsqueeze(1).broadcast_to([128, WIDE, S, 4])
                    )
                    hq = hhn_bf[:, m * S + i0: m * S + i0 + SUPI].rearrange(
                        "p (g i) -> p g i", g=WIDE
                    )
                    in1 = hq.unsqueeze(2).broadcast_to([128, WIDE, S, 4])
                    vv = vt[:].rearrange(
                        "p (g j i) -> p g j i", g=WIDE, i=IGRP
                    )
                    nc.vector.tensor_tensor(vv, in0, in1, op=ALU.max)
                    if fin_after:
                        fin_m(m)
                return go

            def act_thunk(a, sup, gl, fin_after):
                def go():
                    vt = va_t[a][sup % NRING]
                    for ii in range(IGRP):
                        i = sup * SUPI + gl * IGRP + ii
                        dst = vt[:, gl * NFREE:(gl + 1) * NFREE].rearrange(
                            "p (j r) -> p j r", r=4
                        )[:, :, ii]
                        nc.scalar.activation(
                            dst, ht_bA[:, a * S:(a + 1) * S], AF.Relu,
                            bias=hh_fA[:, a * S + i: a * S + i + 1], scale=1.0,
                        )
                    if fin_after:
                        fin_m(N_DVE + a)
                return go

            def make_prod_units(sup, with_fin):
                units = []
                for m in DVE_MS:
                    units.append(pair_thunk(m, sup, with_fin))
                for a in range(N_ACT):
                    for gl in range(WIDE):
                        units.append(
                            act_thunk(a, sup, gl, with_fin and gl == WIDE - 1)
                        )
                return units

            pend = []

            def chain(g):
                sup, gl = g // WIDE, g % WIDE
                ops = pso.tile([C, NFREE], f32, name="ops", tag="ops")
                mlist = DVE_MS + ACT_MS
                for j, m in enumerate(mlist):
                    vt = (vd_t[m][sup % NRING] if m < N_DVE
                          else va_t[m - N_DVE][sup % NRING])
                    nc.tensor.matmul(
                        ops[:], rwr(m), vt[:, gl * NFREE:(gl + 1) * NFREE],
                        start=(j == 0), stop=False,
                    )
                # + P[c, i] broadcast over j via constant selector
                base = (g // 8) * 32
                q = g % 8
                nc.tensor.matmul(
                    ops[:], pT_sb[base:base + 32, :],
                    sel_sb[base:base + 32, q * NFREE:(q + 1) * NFREE],
                    start=False, stop=True,
                )
                pend.append((g, ops))

            def flush_one():
                g0, ops0 = pend.pop(0)
                ostg = ostg_t[g0 % 3]
                src = ops0[:].rearrange("c (j i) -> c i j", i=IGRP)
                dst = ostg[:].rearrange("c (i j) -> c i j", j=S)
                nc.scalar.copy(dst, src)
                nc.sync.dma_start(
                    out[:, g0 * NFREE:(g0 + 1) * NFREE], ostg[:]
                )

            def emit_p():
                # pT[i, c] = -(hhn^T rw)[i, c] = (rw^T hh)^T, DVE-owned m only
                ps = pstmp.tile([128, NFREE], f32, name="tmp", tag="tmp")
                for j, m in enumerate(DVE_MS):
                    nc.tensor.matmul(
                        ps[:S, :C], hhn_bf[:, m * S:(m + 1) * S], rwr(m),
                        start=(j == 0), stop=(j == N_DVE - 1),
                        skip_group_check=True,
                    )
                nc.scalar.mul(pT_sb[:], ps[:S, :C], -1.0)

            def emit_from(q, n):
                for _ in range(n):
                    if q:
                        q.pop(0)()

            def emit_body(do_scan, prod_next_sup0):
                # super w produced in slots [max(0, 2w-4) .. 2w-1]; ring-3
                # parity guarantees writes land after the last chain read of
                # super w-3 (slots 2w-6, 2w-5).
                prod_qs = {
                    w: make_prod_units(w, with_fin=(do_scan and w == NSUP - 1))
                    for w in range(1, NSUP)
                }
                windows = {
                    w: list(range(WIDE * (w - 1), WIDE * w))
                    for w in range(1, NSUP)
                }
                quota = {}
                for w, sl in windows.items():
                    n = len(prod_qs[w])
                    base = n // len(sl)
                    ext = n - base * len(sl)
                    for idx, s_ in enumerate(sl):
                        quota[(w, s_)] = base + (1 if idx < ext else 0)
                sup0_q = make_prod_units(0, False) if prod_next_sup0 else []
                step_i = {"n": 0}
                for slot in range(NSLOT):
                    if do_scan and slot == 12:
                        emit_wsum_bf()
                        emit_corr()
                    if do_scan and slot == 13:
                        phase_a()
                    halves = ([], [])
                    for w in range(1, NSUP):
                        q = quota.get((w, slot))
                        if q:
                            halves[0].append((prod_qs[w], (q + 1) // 2))
                            halves[1].append((prod_qs[w], q // 2))
                    if do_scan and slot < 12:
                        scan_step(step_i["n"])
                        step_i["n"] += 1
                    for qq, n in halves[0]:
                        emit_from(qq, n)
                    if do_scan and slot < 12:
                        scan_step(step_i["n"])
                        step_i["n"] += 1
                    for qq, n in halves[1]:
                        emit_from(qq, n)
                    if prod_next_sup0 and slot >= NSLOT - WIDE:
                        emit_from(sup0_q, 6)
                    chain(slot)
                    if len(pend) > 2:
                        flush_one()
                for w in range(1, NSUP):
                    emit_from(prod_qs[w], 99)
                if prod_next_sup0:
                    emit_from(sup0_q, 99)
                while pend:
                    flush_one()
                if do_scan:
                    emit_p()

            def emit_prologue():
                phase_a()
                for t in range(R):
                    scan_step(t)
                emit_wsum_bf()
                emit_corr()
                for m in range(MT):
                    fin_m(m)
                emit_p()
                phase_a()
                for th in make_prod_units(0, False):
                    th()

            emit_prologue()
            n_bodies = repeat - 1
            if n_bodies >= 4:
                with tc.For_i(0, n_bodies // 4, 1):
                    for _ in range(4):
                        emit_body(do_scan=True, prod_next_sup0=True)
            for _ in range(n_bodies % 4):
                emit_body(do_scan=True, prod_next_sup0=True)
            emit_body(do_scan=False, prod_next_sup0=False)

    nc.finalize()
    return nc


_CACHED = {}


def _prep_in_maps(encoded_text, rel_types_encoded, proj_W, proj_b, rel_W):
    import ml_dtypes

    relw_perm = np.ascontiguousarray(
        np.asarray(rel_W, np.float32).reshape(H3, R, TAG)
        .transpose(0, 2, 1).reshape(H3, C)
    )
    rw_pack = np.zeros((128, MT * C), np.float32)
    for m in range(MT):
        rw_pack[:, m * C:(m + 1) * C] = relw_perm[m * 128:(m + 1) * 128, :]
    rw_pack = rw_pack.astype(ml_dtypes.bfloat16)

    pw = np.asarray(proj_W, np.float32)
    pw_pack = np.zeros((128, 2 * KT * H3), np.float32)
    for kt in range(2 * KT):
        pw_pack[:, kt * H3:(kt + 1) * H3] = pw[kt * 128:(kt + 1) * 128, :]
    pw_pack = pw_pack.astype(ml_dtypes.bfloat16)

    sel = np.zeros((96, 8 * NFREE), np.float32)
    for base in (0, 32, 64):
        for q in range(8):
            for j in range(S):
                for ii in range(IGRP):
                    sel[base + q * IGRP + ii,
                        q * NFREE + j * IGRP + ii] = 1.0
    sel = sel.astype(ml_dtypes.bfloat16)

    in_maps = []
    for i in range(B):
        in_maps.append({
            "enc": np.ascontiguousarray(encoded_text[i], dtype=np.float32),
            "arel": np.ascontiguousarray(
                rel_types_encoded[i], dtype=np.float32
            ),
            "pwbf": pw_pack,
            "projb": np.ascontiguousarray(proj_b, dtype=np.float32),
            "rwbf": rw_pack,
            "sel4": sel,
        })
    return in_maps


def _assemble(results, rel_b):
    outs = []
    for i in range(B):
        o = results[i]["out"].reshape(TAG, R, S, S)
        outs.append(o)
    full = np.stack(outs, axis=0).astype(np.float32)
    if np.any(rel_b):
        relb_perm = np.asarray(rel_b, dtype=np.float32).reshape(R, TAG).T
        full = full + relb_perm[None, :, :, None, None]
    return full


def kernel(encoded_text, rel_types_encoded, proj_W, proj_b, rel_W, rel_b):
    from concourse.bass_utils import run_bass_kernel_spmd

    use_pb = bool(np.any(np.asarray(proj_b)))
    key = (1, use_pb)
    if key not in _CACHED:
        _CACHED[key] = build_nc(repeat=1, use_pb=use_pb)
    in_maps = _prep_in_maps(
        encoded_text, rel_types_encoded, proj_W, proj_b, rel_W
    )
    res = run_bass_kernel_spmd(_CACHED[key], in_maps, list(range(B)))
    return _assemble(res.results, rel_b)
